# revision 22
# baseline (speedup 1.0000x reference)
"""Multi-head attention (dense transformer block) for 8 Trainium2 NeuronCores.

Problem: x [4, 2048, 1024] f32, w_qkv [3072, 1024], w_out [1024, 1024]
  qkv = x @ w_qkv.T ; split q,k,v ; 16 heads x 64 dims
  out = softmax(q k^T / 8) v ; y = out @ w_out.T

Sharding: 8 shards = (batch b in 0..3) x (head-half hh in 0..1).
Each core handles one batch and 8 heads end-to-end: QKV projection
column-split, attention for its 8 heads, out-projection row-split ->
partial y. Host sums the two partial y's per batch. No collectives.

Kernel structure (engines run their instruction streams in order, so the
phases are emitted as a software pipeline over head pairs):

    qk(0) | v | B(0) qk(1) C(0) | B(1) qk(2) C(1) | ... | B(3) C(3)

  - qk(p): q^T,k^T [128, tok] for pair p (fp32r matmuls, rotating bufs)
  - v: value projection -> vaug bf16 [ktok, head, 65] with a ones column
  - B(p): attention. Scores computed transposed per head S^T[ktok, qtok]
    with the two heads PAIRED via PE row-tiling (K=64 at partitions
    0/64) into adjacent PSUM banks; one ScalarE exp ACTIVATE [128, 1024]
    per k-tile covers both heads with the 1/8 scale folded in (softmax
    max-subtraction skipped; scores are O(+-6)). AV matmuls in bf16 with
    M=65: the ones column makes PSUM row 64 the softmax denominators.
    Normalization: DVE reciprocal -> GpSimd partition-broadcast -> DVE
    multiply (PE stays out of the softmax epilogue).
  - C(p): per-pair out-projection (K=128), accumulated into y in DRAM
    (first pair writes, later pairs DMA-accumulate).
"""

import numpy as np

B = 4
NT = 2048          # tokens per batch
E = 1024           # embed dim
H = 16             # heads
DH = 64            # head dim
HD = 512           # head dims per core (8 heads)
N_CORES = 8
SCALE = DH ** -0.5
P = 128

# DVE Schraudolph fast-exp: bf16 bits of exp(SCALE*s) ~= int16(s*FE_S + FE_B)
# (bf16 = 8-bit exponent + 7-bit mantissa; linear-mantissa approx, +-3% rel
# err, bias cancels in softmax). Lets the DVE take half the softmax exps.
import math
FE_S = SCALE * 128.0 / math.log(2.0)
FE_B = 127.0 * 128.0 - 5.5

_cache = {}


def _build(rep=1, ablate=(), mmdt="f32r", loop=False):
    import concourse.mybir as mybir
    import concourse.tile as tile
    from concourse import bacc
    from contextlib import ExitStack

    # dtype scheme: f32r/bf16/fp16 uniform; "mix" = fp16 q/k path + bf16 soft path
    f32 = mybir.dt.float32
    _qk = {"f32r": mybir.dt.float32r, "bf16": mybir.dt.bfloat16,
           "fp16": mybir.dt.float16, "mix": mybir.dt.float16}
    _soft = {"f32r": mybir.dt.bfloat16, "bf16": mybir.dt.bfloat16,
             "fp16": mybir.dt.float16, "mix": mybir.dt.bfloat16}
    f32r = _qk[mmdt]          # q/k-side matmul dtype (x, wq, wk, wv, qT, kT)
    bf16 = _soft[mmdt]        # softmax/out-side dtype (es, vaug, outT, woT)
    in_dt = {"f32r": f32, "bf16": mybir.dt.bfloat16,
             "fp16": mybir.dt.float16, "mix": mybir.dt.float16}[mmdt]
    wo_dt = {"f32r": f32, "bf16": mybir.dt.bfloat16,
             "fp16": mybir.dt.float16, "mix": mybir.dt.bfloat16}[mmdt]
    Exp = mybir.ActivationFunctionType.Exp
    Add = mybir.AluOpType.add

    nc = bacc.Bacc("TRN2", target_bir_lowering=False, debug=False,
                   enable_asserts=False, num_devices=N_CORES)

    xT_ap = nc.dram_tensor("xT", [E, NT], in_dt, kind="ExternalInput").ap()
    wqT_ap = nc.dram_tensor("wqT", [E, HD], in_dt, kind="ExternalInput").ap()
    wkT_ap = nc.dram_tensor("wkT", [E, HD], in_dt, kind="ExternalInput").ap()
    wvT_ap = nc.dram_tensor("wvT", [E, HD], in_dt, kind="ExternalInput").ap()
    woT_ap = nc.dram_tensor("woT", [HD, E], wo_dt, kind="ExternalInput").ap()
    y_ap = nc.dram_tensor("y", [NT, E], f32, kind="ExternalOutput").ap()

    KE = E // P        # 8 contraction tiles over embed
    MQ = HD // P       # 4 partition tiles over head dims = head pairs
    TQ = NT // 512     # 4 query chunks of 512
    TT = NT // P       # 16 token tiles of 128

    from concourse.tile_rust import add_dep_helper

    with tile.TileContext(nc) as tc, ExitStack() as ctx:
        per = ctx.enter_context(tc.tile_pool(name="per", bufs=1))
        qk_pool = ctx.enter_context(tc.tile_pool(name="qk", bufs=2))
        outT_pool = ctx.enter_context(tc.tile_pool(name="ot", bufs=3))
        es_pool = ctx.enter_context(tc.tile_pool(name="es", bufs=3))
        y_pool = ctx.enter_context(tc.tile_pool(name="ysb", bufs=2))
        nrm_pool = ctx.enter_context(tc.tile_pool(name="nrm", bufs=2))
        bcs_pool = ctx.enter_context(tc.tile_pool(name="bcs", bufs=2))
        xT_pool = ctx.enter_context(tc.tile_pool(name="xTp", bufs=2))
        psS = ctx.enter_context(tc.tile_pool(name="psS", bufs=2, space="PSUM"))
        psAV = ctx.enter_context(tc.tile_pool(name="psAV", bufs=2, space="PSUM"))
        psM = ctx.enter_context(tc.tile_pool(name="psM", bufs=2, space="PSUM"))

        # rep-invariant weights (wv first: the value projection runs first)
        wv = per.tile([P, KE, HD], f32r, tag="wv")
        nc.scalar.dma_start(wv[:], wvT_ap.rearrange("(o p) m -> p o m", p=P).bitcast(f32r))
        wq = per.tile([P, KE, HD], f32r, tag="wq")
        nc.scalar.dma_start(wq[:], wqT_ap.rearrange("(o p) m -> p o m", p=P).bitcast(f32r))
        wk = per.tile([P, KE, HD], f32r, tag="wk")
        nc.scalar.dma_start(wk[:], wkT_ap.rearrange("(o p) m -> p o m", p=P).bitcast(f32r))
        woT = per.tile([P, MQ, E], bf16, tag="woT")
        nc.scalar.dma_start(woT[:], woT_ap.rearrange("(o p) e -> p o e", p=P).bitcast(bf16))
        vaug_g = [per.tile([P, 4, 8, DH + 1], bf16, tag=f"vaug{g}", name=f"vaug{g}")
                  for g in range(TT // 4)]
        vaugs = [vaug_g[tt // 4][:, tt % 4] for tt in range(TT)]
        for g in range(TT // 4):
            nc.vector.memset(vaug_g[g][:, :, :, DH:DH + 1], 1.0)

        # Tile does not order DMAs by DRAM range: chain each y region's
        # write/accumulate DMAs explicitly (across pairs and reps).
        y_prev_dma = {}
        # last tq's outproj chunks, deferred past their pair (and, for the
        # final pair, into the next rep's first attention window) so their
        # matmuls never wait on the just-produced softmax epilogue.
        pending_tail = [None]

        def emit_body():
            xTs = []
            xT_src = xT_ap.rearrange("(o p) t -> p o t", p=P).bitcast(f32r)
            for ke in range(KE):
                xk = xT_pool.tile([P, NT], f32r, tag=f"xT{ke}", name=f"xT{ke}")
                nc.sync.dma_start(xk[:], xT_src[:, ke, :])
                xTs.append(xk)

            def emit_qk_group(mq, dst, w, tq, rot=0):
                ps = psM.tile([P, 512], f32, tag="m")
                for i in range(KE):
                    ke = (i + rot) % KE
                    nc.tensor.matmul(ps[:], w[:, ke, mq * P:(mq + 1) * P],
                                     xTs[ke][:, tq * 512:(tq + 1) * 512],
                                     start=(i == 0), stop=(i == KE - 1))
                nc.vector.tensor_copy(dst[:, tq * 512:(tq + 1) * 512], ps[:])

            def alloc_qk(mq):
                qT = qk_pool.tile([P, NT], f32r, tag="qTp", name=f"qT{mq}")
                kT = qk_pool.tile([P, NT], f32r, tag="kTp", name=f"kT{mq}")
                return qT, kT

            def qk_groups(mq, qT, kT):
                for dst, w in ((kT, wk), (qT, wq)):
                    for tq in range(TQ):
                        yield (mq, dst, w, tq)

            def emit_v():
                for tt in range(TT):
                    ps = psM.tile([P, HD], f32, tag="m")
                    for i in range(KE):
                        ke = (i + tt) % KE
                        nc.tensor.matmul(ps[:], xTs[ke][:, tt * P:(tt + 1) * P],
                                         wv[:, ke, :], start=(i == 0), stop=(i == KE - 1))
                    nc.scalar.copy(vaugs[tt][:, :, 0:DH],
                                   ps[:].rearrange("p (h d) -> p h d", h=8))

            def emit_attn_tq(pair, qT, kT, outT, tq):
                qsl = slice(tq * 512, (tq + 1) * 512)
                av0 = psAV.tile([DH + 1, 512], f32, tag="av")
                av1 = psAV.tile([DH + 1, 512], f32, tag="av")

                def emit_av(kt, es):
                    nc.tensor.matmul(av0[:], vaugs[kt][:, 2 * pair, :], es[:, 0, :],
                                     start=(kt == 0), stop=(kt == TT - 1))
                    nc.tensor.matmul(av1[:], vaugs[kt][:, 2 * pair + 1, :], es[:, 1, :],
                                     start=(kt == 0), stop=(kt == TT - 1))

                # AV lags scores/exp by one k-tile so the PE never sits in
                # the scores -> exp -> AV semaphore chain: while ScalarE
                # exps tile kt, the PE already runs scores of kt+1.
                pending = None
                for kt in range(TT):
                    ksl = slice(kt * P, (kt + 1) * P)
                    sps = psS.tile([P, 2, 512], f32, tag="s")
                    nc.tensor.matmul(sps[:, 0, :], kT[0:DH, ksl],
                                     qT[0:DH, qsl], start=True, stop=True)
                    nc.tensor.matmul(sps[:, 1, :], kT[DH:P, ksl],
                                     qT[DH:P, qsl], start=True, stop=True)
                    if "exp" in ablate:
                        continue
                    es = es_pool.tile([P, 2, 512], bf16, tag="es")
                    if bf16 == mybir.dt.bfloat16 and kt % 8 < 5:
                        # split this kt's exp across engines: ScalarE takes
                        # head 0, DVE fast-exp takes head 1 (different PSUM
                        # banks, so the reads run concurrently). Only 5 of
                        # every 8 kt-steps split, equalizing ScalarE and DVE
                        # totals below the PE stream floor.
                        nc.scalar.activation(es[:, 0, :], sps[:, 0, :], Exp,
                                             scale=SCALE)
                        nc.vector.tensor_scalar(
                            es[:, 1, :].bitcast(mybir.dt.int16), sps[:, 1, :],
                            FE_S, FE_B,
                            mybir.AluOpType.mult, mybir.AluOpType.add)
                    else:
                        nc.scalar.activation(es[:], sps[:], Exp, scale=SCALE)
                    if "av" in ablate:
                        continue
                    if pending is not None:
                        emit_av(*pending)
                    pending = (kt, es)
                if "av" not in ablate and "exp" not in ablate:
                    emit_av(*pending)
                if "av" in ablate or "exp" in ablate:
                    return
                for j, av in ((0, av0), (1, av1)):
                    # custom-DVE recip requires matching in/out base
                    # partitions; den sits at PSUM partition 64, so hop it
                    # to partition 0 first (stock copy handles the shift).
                    den = nrm_pool.tile([1, 512], f32, tag="den")
                    nc.vector.tensor_copy(den[:], av[DH:DH + 1, :])
                    recip = nrm_pool.tile([1, 512], f32, tag="recip")
                    nc.vector.reciprocal_approx_fast(recip[:], den[:])
                    bcs = bcs_pool.tile([DH, 512], f32, tag="bcs")
                    nc.gpsimd.partition_broadcast(bcs[:], recip[:])
                    nc.vector.tensor_tensor(
                        outT[j * DH:(j + 1) * DH, qsl],
                        av[0:DH, :], bcs[:], mybir.AluOpType.mult)

            def outproj_chunks(pair, outT):
                # y (+)= outT(pair).T @ woT[pair]; DRAM-side accumulation
                for tt in range(TT):
                    for ec in range(E // 512):
                        yield (pair, outT, tt, ec)

            def emit_outproj_chunk(pair, outT_a, outT_b, tt, ec):
                # two pairs' contributions accumulated in PSUM, then one
                # write (first half) or DMA-accumulate (second half).
                # ps lives in psM (not psAV) so outproj matmuls never wait
                # on the softmax epilogue's reads of the av tiles.
                esl = slice(ec * 512, (ec + 1) * 512)
                ps = psM.tile([P, 512], f32, tag="m")
                nc.tensor.matmul(ps[:], outT_a[:, tt * P:(tt + 1) * P],
                                 woT[:, pair - 1, esl], start=True, stop=False)
                nc.tensor.matmul(ps[:], outT_b[:, tt * P:(tt + 1) * P],
                                 woT[:, pair, esl], start=False, stop=True)
                ysb = y_pool.tile([P, 512], f32, tag="ysb")
                nc.vector.tensor_copy(ysb[:], ps[:])
                if pair == 1:
                    dma = nc.sync.dma_start(y_ap[tt * P:(tt + 1) * P, esl], ysb[:])
                else:
                    dma = nc.gpsimd.dma_start(y_ap[tt * P:(tt + 1) * P, esl],
                                              ysb[:], accum_op=Add)
                if (tt, ec) in y_prev_dma:
                    add_dep_helper(dma.ins, y_prev_dma[(tt, ec)].ins,
                                   reason="y accumulation order")
                y_prev_dma[(tt, ec)] = dma

            def drain(it, n):
                for _ in range(n):
                    args = next(it, None)
                    if args is None:
                        return
                    if len(args) == 4 and isinstance(args[0], int) and args[0] < MQ and not hasattr(args[1], "shape"):
                        emit_qk_group(*args)
                    else:
                        emit_outproj_chunk(*args)

            # software pipeline over head pairs:
            #   v | qk(0) | B(0)+qk(1)+C(0) | B(1)+qk(2)+C(1) | ... | B(3)+C(3)
            emit_v()
            qT, kT = alloc_qk(0)
            for gi, g in enumerate(qk_groups(0, qT, kT)):
                emit_qk_group(*g, rot=gi)
            prev_outT = None
            for pair in range(MQ):
                outT = outT_pool.tile([P, NT], bf16, tag="outT", name=f"outT{pair}")
                if pair + 1 < MQ:
                    nqT, nkT = alloc_qk(pair + 1)
                    qk_iter = qk_groups(pair + 1, nqT, nkT)
                else:
                    nqT = nkT = None
                    qk_iter = iter(())
                for tq in range(TQ):
                    if "scores" not in ablate:
                        emit_attn_tq(pair, qT, kT, outT, tq)
                    if tq == 0 and pending_tail[0] is not None:
                        pending_tail[0]()
                        pending_tail[0] = None
                    for _ in range(2):
                        g = next(qk_iter, None)
                        if g is not None:
                            emit_qk_group(*g)
                    if "outproj" in ablate or pair % 2 == 0:
                        continue
                    # out-projection lags attention by one tq chunk so the
                    # PE never waits on the softmax epilogue of the chunk
                    # it just produced.
                    if tq == 0:
                        continue
                    for tt in range((tq - 1) * 4, tq * 4):
                        for ec in range(E // 512):
                            emit_outproj_chunk(pair, prev_outT, outT, tt, ec)
                if "outproj" not in ablate and pair % 2 == 1:
                    def _tail(pa=pair, oa=prev_outT, ob=outT):
                        for tt in range(12, 16):
                            for ec in range(E // 512):
                                emit_outproj_chunk(pa, oa, ob, tt, ec)
                    pending_tail[0] = _tail
                prev_outT = outT
                qT, kT = nqT, nkT

        if loop:
            with tc.For_i(0, rep, 1):
                emit_body()
        else:
            for _ in range(rep):
                emit_body()
        if pending_tail[0] is not None:
            pending_tail[0]()
            pending_tail[0] = None

    nc.compile()
    return nc


MMDT = "bf16"


def _get_nc(rep=1, ablate=(), mmdt=None):
    mmdt = mmdt or MMDT
    key = ("nc", rep, tuple(sorted(ablate)), mmdt)
    if key not in _cache:
        _cache[key] = _build(rep, ablate, mmdt)
    return _cache[key]


def make_in_maps(x, w_qkv, w_out, mmdt=None):
    import ml_dtypes
    mmdt = mmdt or MMDT
    dt = {"f32r": np.float32, "bf16": ml_dtypes.bfloat16,
          "fp16": np.float16, "mix": np.float16}[mmdt]
    wo_np = {"f32r": np.float32, "bf16": ml_dtypes.bfloat16,
             "fp16": np.float16, "mix": ml_dtypes.bfloat16}[mmdt]
    x = np.asarray(x, dtype=np.float32).astype(dt)
    w_qkv = np.asarray(w_qkv, dtype=np.float32).astype(dt)
    w_out = np.asarray(w_out, dtype=np.float32).astype(wo_np)
    in_maps = []
    for c in range(N_CORES):
        b, hh = divmod(c, 2)
        hsl = slice(hh * HD, (hh + 1) * HD)
        in_maps.append({
            "xT": np.ascontiguousarray(x[b].T),
            "wqT": np.ascontiguousarray(w_qkv[0 * E:1 * E][hsl].T),
            "wkT": np.ascontiguousarray(w_qkv[1 * E:2 * E][hsl].T),
            "wvT": np.ascontiguousarray(w_qkv[2 * E:3 * E][hsl].T),
            "woT": np.ascontiguousarray(w_out[:, hsl].T),
        })
    return in_maps


def combine_outputs(results):
    y = np.empty((B, NT, E), dtype=np.float32)
    for b in range(B):
        y[b] = results[2 * b]["y"] + results[2 * b + 1]["y"]
    return y


def kernel(x, w_qkv, w_out):
    from concourse.bass_utils import run_bass_kernel_spmd
    nc = _get_nc()
    in_maps = make_in_maps(x, w_qkv, w_out)
    res = run_bass_kernel_spmd(nc, in_maps, core_ids=list(range(N_CORES)))
    return combine_outputs(res.results)



# revision 25
# speedup vs baseline: 1.0567x; 1.0567x over previous
"""Multi-head attention (dense transformer block) for 8 Trainium2 NeuronCores.

Problem: x [4, 2048, 1024] f32, w_qkv [3072, 1024], w_out [1024, 1024]
  qkv = x @ w_qkv.T ; split q,k,v ; 16 heads x 64 dims
  out = softmax(q k^T / 8) v ; y = out @ w_out.T

Sharding: 8 shards = (batch b in 0..3) x (head-half hh in 0..1).
Each core handles one batch and 8 heads end-to-end: QKV projection
column-split, attention for its 8 heads, out-projection row-split ->
partial y. Host sums the two partial y's per batch. No collectives.

Kernel structure (engines run their instruction streams in order, so the
phases are emitted as a software pipeline over head pairs):

    qk(0) | v | B(0) qk(1) C(0) | B(1) qk(2) C(1) | ... | B(3) C(3)

  - qk(p): q^T,k^T [128, tok] for pair p (fp32r matmuls, rotating bufs)
  - v: value projection -> vaug bf16 [ktok, head, 65] with a ones column
  - B(p): attention. Scores computed transposed per head S^T[ktok, qtok]
    with the two heads PAIRED via PE row-tiling (K=64 at partitions
    0/64) into adjacent PSUM banks; one ScalarE exp ACTIVATE [128, 1024]
    per k-tile covers both heads with the 1/8 scale folded in (softmax
    max-subtraction skipped; scores are O(+-6)). AV matmuls in bf16 with
    M=65: the ones column makes PSUM row 64 the softmax denominators.
    Normalization: DVE reciprocal -> GpSimd partition-broadcast -> DVE
    multiply (PE stays out of the softmax epilogue).
  - C(p): per-pair out-projection (K=128), accumulated into y in DRAM
    (first pair writes, later pairs DMA-accumulate).
"""

import numpy as np

B = 4
NT = 2048          # tokens per batch
E = 1024           # embed dim
H = 16             # heads
DH = 64            # head dim
HD = 512           # head dims per core (8 heads)
N_CORES = 8
SCALE = DH ** -0.5
P = 128

# DVE Schraudolph fast-exp: bf16 bits of exp(SCALE*s) ~= int16(s*FE_S + FE_B)
# (bf16 = 8-bit exponent + 7-bit mantissa; linear-mantissa approx, +-3% rel
# err, bias cancels in softmax). Lets the DVE take half the softmax exps.
import math
FE_S = SCALE * 128.0 / math.log(2.0)
FE_B = 127.0 * 128.0 - 5.5

_cache = {}


def _build(rep=1, ablate=(), mmdt="f32r", loop=False):
    import concourse.mybir as mybir
    import concourse.tile as tile
    from concourse import bacc
    from contextlib import ExitStack

    # dtype scheme: f32r/bf16/fp16 uniform; "mix" = fp16 q/k path + bf16 soft path
    f32 = mybir.dt.float32
    _qk = {"f32r": mybir.dt.float32r, "bf16": mybir.dt.bfloat16,
           "fp16": mybir.dt.float16, "mix": mybir.dt.float16}
    _soft = {"f32r": mybir.dt.bfloat16, "bf16": mybir.dt.bfloat16,
             "fp16": mybir.dt.float16, "mix": mybir.dt.bfloat16}
    f32r = _qk[mmdt]          # q/k-side matmul dtype (x, wq, wk, wv, qT, kT)
    bf16 = _soft[mmdt]        # softmax/out-side dtype (es, vaug, outT, woT)
    in_dt = {"f32r": f32, "bf16": mybir.dt.bfloat16,
             "fp16": mybir.dt.float16, "mix": mybir.dt.float16}[mmdt]
    wo_dt = {"f32r": f32, "bf16": mybir.dt.bfloat16,
             "fp16": mybir.dt.float16, "mix": mybir.dt.bfloat16}[mmdt]
    Exp = mybir.ActivationFunctionType.Exp
    Add = mybir.AluOpType.add

    nc = bacc.Bacc("TRN2", target_bir_lowering=False, debug=False,
                   enable_asserts=False, num_devices=N_CORES)

    xT_ap = nc.dram_tensor("xT", [E, NT], in_dt, kind="ExternalInput").ap()
    wqT_ap = nc.dram_tensor("wqT", [E, HD], in_dt, kind="ExternalInput").ap()
    wkT_ap = nc.dram_tensor("wkT", [E, HD], in_dt, kind="ExternalInput").ap()
    wvT_ap = nc.dram_tensor("wvT", [E, HD], in_dt, kind="ExternalInput").ap()
    woT_ap = nc.dram_tensor("woT", [HD, E], wo_dt, kind="ExternalInput").ap()
    y_ap = nc.dram_tensor("y", [NT, E], f32, kind="ExternalOutput").ap()

    KE = E // P        # 8 contraction tiles over embed
    MQ = HD // P       # 4 partition tiles over head dims = head pairs
    TQ = NT // 512     # 4 query chunks of 512
    TT = NT // P       # 16 token tiles of 128

    from concourse.tile_rust import add_dep_helper

    with tile.TileContext(nc) as tc, ExitStack() as ctx:
        per = ctx.enter_context(tc.tile_pool(name="per", bufs=1))
        qk_pool = ctx.enter_context(tc.tile_pool(name="qk", bufs=2))
        outT_pool = ctx.enter_context(tc.tile_pool(name="ot", bufs=3))
        es_pool = ctx.enter_context(tc.tile_pool(name="es", bufs=3))
        y_pool = ctx.enter_context(tc.tile_pool(name="ysb", bufs=2))
        nrm_pool = ctx.enter_context(tc.tile_pool(name="nrm", bufs=2))
        bcs_pool = ctx.enter_context(tc.tile_pool(name="bcs", bufs=2))
        xT_pool = ctx.enter_context(tc.tile_pool(name="xTp", bufs=2))
        psS = ctx.enter_context(tc.tile_pool(name="psS", bufs=2, space="PSUM"))
        psAV = ctx.enter_context(tc.tile_pool(name="psAV", bufs=2, space="PSUM"))
        psM = ctx.enter_context(tc.tile_pool(name="psM", bufs=2, space="PSUM"))

        # rep-invariant weights (wv first: the value projection runs first)
        wv = per.tile([P, KE, HD], f32r, tag="wv")
        nc.scalar.dma_start(wv[:], wvT_ap.rearrange("(o p) m -> p o m", p=P).bitcast(f32r))
        wq = per.tile([P, KE, HD], f32r, tag="wq")
        nc.scalar.dma_start(wq[:], wqT_ap.rearrange("(o p) m -> p o m", p=P).bitcast(f32r))
        wk = per.tile([P, KE, HD], f32r, tag="wk")
        nc.scalar.dma_start(wk[:], wkT_ap.rearrange("(o p) m -> p o m", p=P).bitcast(f32r))
        woT = per.tile([P, MQ, E], bf16, tag="woT")
        nc.scalar.dma_start(woT[:], woT_ap.rearrange("(o p) e -> p o e", p=P).bitcast(bf16))
        vaug_g = [per.tile([P, 4, 8, DH + 1], bf16, tag=f"vaug{g}", name=f"vaug{g}")
                  for g in range(TT // 4)]
        vaugs = [vaug_g[tt // 4][:, tt % 4] for tt in range(TT)]
        for g in range(TT // 4):
            nc.vector.memset(vaug_g[g][:, :, :, DH:DH + 1], 1.0)

        # Tile does not order DMAs by DRAM range: chain each y region's
        # write/accumulate DMAs explicitly (across pairs and reps).
        y_prev_dma = {}
        # last tq's outproj chunks, deferred past their pair (and, for the
        # final pair, into the next rep's first attention window) so their
        # matmuls never wait on the just-produced softmax epilogue.
        pending_tail = [None]

        def emit_body():
            xTs = []
            xT_src = xT_ap.rearrange("(o p) t -> p o t", p=P).bitcast(f32r)
            for ke in range(KE):
                xk = xT_pool.tile([P, NT], f32r, tag=f"xT{ke}", name=f"xT{ke}")
                nc.sync.dma_start(xk[:], xT_src[:, ke, :])
                xTs.append(xk)

            def emit_qk_group(mq, dst, w, tq, rot=0):
                ps = psM.tile([P, 512], f32, tag="m")
                for i in range(KE):
                    ke = (i + rot) % KE
                    nc.tensor.matmul(ps[:], w[:, ke, mq * P:(mq + 1) * P],
                                     xTs[ke][:, tq * 512:(tq + 1) * 512],
                                     start=(i == 0), stop=(i == KE - 1))
                nc.vector.tensor_copy(dst[:, tq * 512:(tq + 1) * 512], ps[:])

            def alloc_qk(mq):
                qT = qk_pool.tile([P, NT], f32r, tag="qTp", name=f"qT{mq}")
                kT = qk_pool.tile([P, NT], f32r, tag="kTp", name=f"kT{mq}")
                return qT, kT

            def qk_groups(mq, qT, kT):
                for dst, w in ((kT, wk), (qT, wq)):
                    for tq in range(TQ):
                        yield (mq, dst, w, tq)

            def emit_v():
                for tt in range(TT):
                    ps = psM.tile([P, HD], f32, tag="m")
                    for i in range(KE):
                        ke = (i + tt) % KE
                        nc.tensor.matmul(ps[:], xTs[ke][:, tt * P:(tt + 1) * P],
                                         wv[:, ke, :], start=(i == 0), stop=(i == KE - 1))
                    nc.scalar.copy(vaugs[tt][:, :, 0:DH],
                                   ps[:].rearrange("p (h d) -> p h d", h=8))

            def emit_attn_tq(pair, qT, kT, outT, tq, filler=()):
                filler = list(filler)
                qsl = slice(tq * 512, (tq + 1) * 512)
                av0 = psAV.tile([DH + 1, 512], f32, tag="av")
                av1 = psAV.tile([DH + 1, 512], f32, tag="av")

                def emit_av(kt, es):
                    nc.tensor.matmul(av0[:], vaugs[kt][:, 2 * pair, :], es[:, 0, :],
                                     start=(kt == 0), stop=(kt == TT - 1))
                    nc.tensor.matmul(av1[:], vaugs[kt][:, 2 * pair + 1, :], es[:, 1, :],
                                     start=(kt == 0), stop=(kt == TT - 1))

                # AV lags scores/exp by one k-tile so the PE never sits in
                # the scores -> exp -> AV semaphore chain: while ScalarE
                # exps tile kt, the PE already runs scores of kt+1.
                pending = None
                for kt in range(TT):
                    # drain filler work (qk groups / out-projection chunks)
                    # spread across the kt loop: the exp-gated PE bubbles
                    # absorb the matmuls and the DVE copies interleave with
                    # the loop instead of ganging up at tq boundaries.
                    if filler:
                        for _ in range(-(-len(filler) // (TT - kt))):
                            filler.pop(0)()
                    ksl = slice(kt * P, (kt + 1) * P)
                    sps = psS.tile([P, 2, 512], f32, tag="s")
                    nc.tensor.matmul(sps[:, 0, :], kT[0:DH, ksl],
                                     qT[0:DH, qsl], start=True, stop=True)
                    nc.tensor.matmul(sps[:, 1, :], kT[DH:P, ksl],
                                     qT[DH:P, qsl], start=True, stop=True)
                    if "exp" in ablate:
                        continue
                    es = es_pool.tile([P, 2, 512], bf16, tag="es")
                    nc.scalar.activation(es[:], sps[:], Exp, scale=SCALE)
                    if "av" in ablate:
                        continue
                    if pending is not None:
                        emit_av(*pending)
                    pending = (kt, es)
                for f in filler:
                    f()
                if "av" not in ablate and "exp" not in ablate:
                    emit_av(*pending)
                if "av" in ablate or "exp" in ablate:
                    return
                for j, av in ((0, av0), (1, av1)):
                    # custom-DVE recip requires matching in/out base
                    # partitions; den sits at PSUM partition 64, so hop it
                    # to partition 0 first (stock copy handles the shift).
                    den = nrm_pool.tile([1, 512], f32, tag="den")
                    nc.vector.tensor_copy(den[:], av[DH:DH + 1, :])
                    recip = nrm_pool.tile([1, 512], f32, tag="recip")
                    nc.vector.reciprocal_approx_fast(recip[:], den[:])
                    bcs = bcs_pool.tile([DH, 512], f32, tag="bcs")
                    nc.gpsimd.partition_broadcast(bcs[:], recip[:])
                    nc.vector.tensor_tensor(
                        outT[j * DH:(j + 1) * DH, qsl],
                        av[0:DH, :], bcs[:], mybir.AluOpType.mult)

            def outproj_chunks(pair, outT):
                # y (+)= outT(pair).T @ woT[pair]; DRAM-side accumulation
                for tt in range(TT):
                    for ec in range(E // 512):
                        yield (pair, outT, tt, ec)

            def emit_outproj_chunk(pair, outT_a, outT_b, tt, ec):
                # two pairs' contributions accumulated in PSUM, then one
                # write (first half) or DMA-accumulate (second half).
                # ps lives in psM (not psAV) so outproj matmuls never wait
                # on the softmax epilogue's reads of the av tiles.
                esl = slice(ec * 512, (ec + 1) * 512)
                ps = psM.tile([P, 512], f32, tag="m")
                nc.tensor.matmul(ps[:], outT_a[:, tt * P:(tt + 1) * P],
                                 woT[:, pair - 1, esl], start=True, stop=False)
                nc.tensor.matmul(ps[:], outT_b[:, tt * P:(tt + 1) * P],
                                 woT[:, pair, esl], start=False, stop=True)
                ysb = y_pool.tile([P, 512], f32, tag="ysb")
                nc.vector.tensor_copy(ysb[:], ps[:])
                if pair == 1:
                    dma = nc.sync.dma_start(y_ap[tt * P:(tt + 1) * P, esl], ysb[:])
                else:
                    dma = nc.gpsimd.dma_start(y_ap[tt * P:(tt + 1) * P, esl],
                                              ysb[:], accum_op=Add)
                if (tt, ec) in y_prev_dma:
                    add_dep_helper(dma.ins, y_prev_dma[(tt, ec)].ins,
                                   reason="y accumulation order")
                y_prev_dma[(tt, ec)] = dma

            def drain(it, n):
                for _ in range(n):
                    args = next(it, None)
                    if args is None:
                        return
                    if len(args) == 4 and isinstance(args[0], int) and args[0] < MQ and not hasattr(args[1], "shape"):
                        emit_qk_group(*args)
                    else:
                        emit_outproj_chunk(*args)

            # software pipeline over head pairs:
            #   v | qk(0) | B(0)+qk(1)+C(0) | B(1)+qk(2)+C(1) | ... | B(3)+C(3)
            emit_v()
            qT, kT = alloc_qk(0)
            for gi, g in enumerate(qk_groups(0, qT, kT)):
                emit_qk_group(*g, rot=gi)
            prev_outT = None
            for pair in range(MQ):
                outT = outT_pool.tile([P, NT], bf16, tag="outT", name=f"outT{pair}")
                if pair + 1 < MQ:
                    nqT, nkT = alloc_qk(pair + 1)
                    qk_iter = qk_groups(pair + 1, nqT, nkT)
                else:
                    nqT = nkT = None
                    qk_iter = iter(())
                for tq in range(TQ):
                    filler = []
                    if tq == 0 and pending_tail[0] is not None:
                        filler += pending_tail[0]
                        pending_tail[0] = None
                    for _ in range(2):
                        g = next(qk_iter, None)
                        if g is not None:
                            filler.append(lambda g=g: emit_qk_group(*g))
                    if "outproj" not in ablate and pair % 2 == 1 and tq >= 1:
                        # out-projection lags attention by one tq chunk so
                        # its matmuls never wait on the epilogue of the
                        # chunk just produced.
                        for tt in range((tq - 1) * 4, tq * 4):
                            for ec in range(E // 512):
                                filler.append(
                                    lambda p=pair, oa=prev_outT, ob=outT,
                                    tt=tt, ec=ec:
                                    emit_outproj_chunk(p, oa, ob, tt, ec))
                    if "scores" not in ablate:
                        emit_attn_tq(pair, qT, kT, outT, tq, filler)
                    else:
                        for f in filler:
                            f()
                if "outproj" not in ablate and pair % 2 == 1:
                    pending_tail[0] = [
                        lambda p=pair, oa=prev_outT, ob=outT, tt=tt, ec=ec:
                        emit_outproj_chunk(p, oa, ob, tt, ec)
                        for tt in range(12, 16) for ec in range(E // 512)]
                prev_outT = outT
                qT, kT = nqT, nkT

        if loop:
            with tc.For_i(0, rep, 1):
                emit_body()
        else:
            for _ in range(rep):
                emit_body()
        if pending_tail[0] is not None:
            for f in pending_tail[0]:
                f()
            pending_tail[0] = None

    nc.compile()
    return nc


MMDT = "bf16"


def _get_nc(rep=1, ablate=(), mmdt=None):
    mmdt = mmdt or MMDT
    key = ("nc", rep, tuple(sorted(ablate)), mmdt)
    if key not in _cache:
        _cache[key] = _build(rep, ablate, mmdt)
    return _cache[key]


def make_in_maps(x, w_qkv, w_out, mmdt=None):
    import ml_dtypes
    mmdt = mmdt or MMDT
    dt = {"f32r": np.float32, "bf16": ml_dtypes.bfloat16,
          "fp16": np.float16, "mix": np.float16}[mmdt]
    wo_np = {"f32r": np.float32, "bf16": ml_dtypes.bfloat16,
             "fp16": np.float16, "mix": ml_dtypes.bfloat16}[mmdt]
    x = np.asarray(x, dtype=np.float32).astype(dt)
    w_qkv = np.asarray(w_qkv, dtype=np.float32).astype(dt)
    w_out = np.asarray(w_out, dtype=np.float32).astype(wo_np)
    in_maps = []
    for c in range(N_CORES):
        b, hh = divmod(c, 2)
        hsl = slice(hh * HD, (hh + 1) * HD)
        in_maps.append({
            "xT": np.ascontiguousarray(x[b].T),
            "wqT": np.ascontiguousarray(w_qkv[0 * E:1 * E][hsl].T),
            "wkT": np.ascontiguousarray(w_qkv[1 * E:2 * E][hsl].T),
            "wvT": np.ascontiguousarray(w_qkv[2 * E:3 * E][hsl].T),
            "woT": np.ascontiguousarray(w_out[:, hsl].T),
        })
    return in_maps


def combine_outputs(results):
    y = np.empty((B, NT, E), dtype=np.float32)
    for b in range(B):
        y[b] = results[2 * b]["y"] + results[2 * b + 1]["y"]
    return y


def kernel(x, w_qkv, w_out):
    from concourse.bass_utils import run_bass_kernel_spmd
    nc = _get_nc()
    in_maps = make_in_maps(x, w_qkv, w_out)
    res = run_bass_kernel_spmd(nc, in_maps, core_ids=list(range(N_CORES)))
    return combine_outputs(res.results)



# revision 30
# speedup vs baseline: 1.1083x; 1.0488x over previous
"""Multi-head attention (dense transformer block) for 8 Trainium2 NeuronCores.

Problem: x [4, 2048, 1024] f32, w_qkv [3072, 1024], w_out [1024, 1024]
  qkv = x @ w_qkv.T ; split q,k,v ; 16 heads x 64 dims
  out = softmax(q k^T / 8) v ; y = out @ w_out.T

Sharding: 8 shards = (batch b in 0..3) x (head-half hh in 0..1).
Each core handles one batch and 8 heads end-to-end: QKV projection
column-split, attention for its 8 heads, out-projection row-split ->
partial y. Host sums the two partial y's per batch. No collectives.

Kernel structure (engines run their instruction streams in order, so the
phases are emitted as a software pipeline over head pairs):

    qk(0) | v | B(0) qk(1) C(0) | B(1) qk(2) C(1) | ... | B(3) C(3)

  - qk(p): q^T,k^T [128, tok] for pair p (fp32r matmuls, rotating bufs)
  - v: value projection -> vaug bf16 [ktok, head, 65] with a ones column
  - B(p): attention. Scores computed transposed per head S^T[ktok, qtok]
    with the two heads PAIRED via PE row-tiling (K=64 at partitions
    0/64) into adjacent PSUM banks; one ScalarE exp ACTIVATE [128, 1024]
    per k-tile covers both heads with the 1/8 scale folded in (softmax
    max-subtraction skipped; scores are O(+-6)). AV matmuls in bf16 with
    M=65: the ones column makes PSUM row 64 the softmax denominators.
    Normalization: DVE reciprocal -> GpSimd partition-broadcast -> DVE
    multiply (PE stays out of the softmax epilogue).
  - C(p): per-pair out-projection (K=128), accumulated into y in DRAM
    (first pair writes, later pairs DMA-accumulate).
"""

import numpy as np

B = 4
NT = 2048          # tokens per batch
E = 1024           # embed dim
H = 16             # heads
DH = 64            # head dim
HD = 512           # head dims per core (8 heads)
N_CORES = 8
SCALE = DH ** -0.5
P = 128

# DVE Schraudolph fast-exp: bf16 bits of exp(SCALE*s) ~= int16(s*FE_S + FE_B)
# (bf16 = 8-bit exponent + 7-bit mantissa; linear-mantissa approx, +-3% rel
# err, bias cancels in softmax). Lets the DVE take half the softmax exps.
import math
FE_S = SCALE * 128.0 / math.log(2.0)
FE_B = 127.0 * 128.0 - 5.5

_cache = {}


def _build(rep=1, ablate=(), mmdt="f32r", loop=False):
    import concourse.mybir as mybir
    import concourse.tile as tile
    from concourse import bacc
    from contextlib import ExitStack

    # dtype scheme: f32r/bf16/fp16 uniform; "mix" = fp16 q/k path + bf16 soft path
    f32 = mybir.dt.float32
    _qk = {"f32r": mybir.dt.float32r, "bf16": mybir.dt.bfloat16,
           "fp16": mybir.dt.float16, "mix": mybir.dt.float16}
    _soft = {"f32r": mybir.dt.bfloat16, "bf16": mybir.dt.bfloat16,
             "fp16": mybir.dt.float16, "mix": mybir.dt.bfloat16}
    f32r = _qk[mmdt]          # q/k-side matmul dtype (x, wq, wk, wv, qT, kT)
    bf16 = _soft[mmdt]        # softmax/out-side dtype (es, vaug, outT, woT)
    in_dt = {"f32r": f32, "bf16": mybir.dt.bfloat16,
             "fp16": mybir.dt.float16, "mix": mybir.dt.float16}[mmdt]
    wo_dt = {"f32r": f32, "bf16": mybir.dt.bfloat16,
             "fp16": mybir.dt.float16, "mix": mybir.dt.bfloat16}[mmdt]
    Exp = mybir.ActivationFunctionType.Exp
    Add = mybir.AluOpType.add

    nc = bacc.Bacc("TRN2", target_bir_lowering=False, debug=False,
                   enable_asserts=False, num_devices=N_CORES)

    xT_ap = nc.dram_tensor("xT", [E, NT], in_dt, kind="ExternalInput").ap()
    wqT_ap = nc.dram_tensor("wqT", [E, HD], in_dt, kind="ExternalInput").ap()
    wkT_ap = nc.dram_tensor("wkT", [E, HD], in_dt, kind="ExternalInput").ap()
    wvT_ap = nc.dram_tensor("wvT", [E, HD], in_dt, kind="ExternalInput").ap()
    woT_ap = nc.dram_tensor("woT", [HD, E], wo_dt, kind="ExternalInput").ap()
    y_ap = nc.dram_tensor("y", [NT, E], f32, kind="ExternalOutput").ap()

    KE = E // P        # 8 contraction tiles over embed
    MQ = HD // P       # 4 partition tiles over head dims = head pairs
    TQ = NT // 512     # 4 query chunks of 512
    TT = NT // P       # 16 token tiles of 128

    from concourse.tile_rust import add_dep_helper

    with tile.TileContext(nc) as tc, ExitStack() as ctx:
        per = ctx.enter_context(tc.tile_pool(name="per", bufs=1))
        qk_pool = ctx.enter_context(tc.tile_pool(name="qk", bufs=2))
        outT_pool = ctx.enter_context(tc.tile_pool(name="ot", bufs=5))
        es_pool = ctx.enter_context(tc.tile_pool(name="es", bufs=3))
        y_pool = ctx.enter_context(tc.tile_pool(name="ysb", bufs=2))
        nrm_pool = ctx.enter_context(tc.tile_pool(name="nrm", bufs=2))
        bcs_pool = ctx.enter_context(tc.tile_pool(name="bcs", bufs=2))
        xT_pool = ctx.enter_context(tc.tile_pool(name="xTp", bufs=2))
        psS = ctx.enter_context(tc.tile_pool(name="psS", bufs=2, space="PSUM"))
        psAV = ctx.enter_context(tc.tile_pool(name="psAV", bufs=2, space="PSUM"))
        psM = ctx.enter_context(tc.tile_pool(name="psM", bufs=2, space="PSUM"))

        # rep-invariant weights (wv first: the value projection runs first)
        wv = per.tile([P, KE, HD], f32r, tag="wv")
        nc.scalar.dma_start(wv[:], wvT_ap.rearrange("(o p) m -> p o m", p=P).bitcast(f32r))
        wq = per.tile([P, KE, HD], f32r, tag="wq")
        nc.scalar.dma_start(wq[:], wqT_ap.rearrange("(o p) m -> p o m", p=P).bitcast(f32r))
        wk = per.tile([P, KE, HD], f32r, tag="wk")
        nc.scalar.dma_start(wk[:], wkT_ap.rearrange("(o p) m -> p o m", p=P).bitcast(f32r))
        woT = per.tile([P, MQ, E], bf16, tag="woT")
        nc.scalar.dma_start(woT[:], woT_ap.rearrange("(o p) e -> p o e", p=P).bitcast(bf16))
        vaug_g = [per.tile([P, 4, 8, DH + 1], bf16, tag=f"vaug{g}", name=f"vaug{g}")
                  for g in range(TT // 4)]
        vaugs = [vaug_g[tt // 4][:, tt % 4] for tt in range(TT)]
        for g in range(TT // 4):
            nc.vector.memset(vaug_g[g][:, :, :, DH:DH + 1], 1.0)

        # Tile does not order DMAs by DRAM range: chain each y region's
        # write DMAs explicitly across reps.
        y_prev_dma = {}
        # pending out-projection chunk emitters; chunks are queued once all
        # four pairs' outT tokens for a tq window are complete, and drained
        # as kt-loop filler in later windows (crossing pair and rep
        # boundaries) so their matmuls never wait on a fresh epilogue.
        op_queue = []

        def emit_body():
            xTs = []
            xT_src = xT_ap.rearrange("(o p) t -> p o t", p=P).bitcast(f32r)
            for ke in range(KE):
                xk = xT_pool.tile([P, NT], f32r, tag=f"xT{ke}", name=f"xT{ke}")
                nc.sync.dma_start(xk[:], xT_src[:, ke, :])
                xTs.append(xk)

            def emit_qk_group(mq, dst, w, tq, rot=0):
                ps = psM.tile([P, 512], f32, tag="m")
                for i in range(KE):
                    ke = (i + rot) % KE
                    nc.tensor.matmul(ps[:], w[:, ke, mq * P:(mq + 1) * P],
                                     xTs[ke][:, tq * 512:(tq + 1) * 512],
                                     start=(i == 0), stop=(i == KE - 1))
                nc.vector.tensor_copy(dst[:, tq * 512:(tq + 1) * 512], ps[:])

            def alloc_qk(mq):
                qT = qk_pool.tile([P, NT], f32r, tag="qTp", name=f"qT{mq}")
                kT = qk_pool.tile([P, NT], f32r, tag="kTp", name=f"kT{mq}")
                return qT, kT

            def qk_groups(mq, qT, kT):
                for dst, w in ((kT, wk), (qT, wq)):
                    for tq in range(TQ):
                        yield (mq, dst, w, tq)

            def emit_v():
                for tt in range(TT):
                    ps = psM.tile([P, HD], f32, tag="m")
                    for i in range(KE):
                        ke = (i + tt) % KE
                        nc.tensor.matmul(ps[:], xTs[ke][:, tt * P:(tt + 1) * P],
                                         wv[:, ke, :], start=(i == 0), stop=(i == KE - 1))
                    nc.scalar.copy(vaugs[tt][:, :, 0:DH],
                                   ps[:].rearrange("p (h d) -> p h d", h=8))

            def emit_attn_tq(pair, qT, kT, outT, tq, filler=()):
                filler = list(filler)
                qsl = slice(tq * 512, (tq + 1) * 512)
                av0 = psAV.tile([DH + 1, 512], f32, tag="av")
                av1 = psAV.tile([DH + 1, 512], f32, tag="av")

                def emit_av(kt, es):
                    nc.tensor.matmul(av0[:], vaugs[kt][:, 2 * pair, :], es[:, 0, :],
                                     start=(kt == 0), stop=(kt == TT - 1))
                    nc.tensor.matmul(av1[:], vaugs[kt][:, 2 * pair + 1, :], es[:, 1, :],
                                     start=(kt == 0), stop=(kt == TT - 1))

                # AV lags scores/exp by one k-tile so the PE never sits in
                # the scores -> exp -> AV semaphore chain: while ScalarE
                # exps tile kt, the PE already runs scores of kt+1.
                pending = None
                for kt in range(TT):
                    # drain filler work (qk groups / out-projection chunks)
                    # spread across the kt loop: the exp-gated PE bubbles
                    # absorb the matmuls and the DVE copies interleave with
                    # the loop instead of ganging up at tq boundaries.
                    if filler:
                        for _ in range(-(-len(filler) // (TT - kt))):
                            filler.pop(0)()
                    ksl = slice(kt * P, (kt + 1) * P)
                    sps = psS.tile([P, 2, 512], f32, tag="s")
                    nc.tensor.matmul(sps[:, 0, :], kT[0:DH, ksl],
                                     qT[0:DH, qsl], start=True, stop=True)
                    nc.tensor.matmul(sps[:, 1, :], kT[DH:P, ksl],
                                     qT[DH:P, qsl], start=True, stop=True)
                    if "exp" in ablate:
                        continue
                    es = es_pool.tile([P, 2, 512], bf16, tag="es")
                    nc.scalar.activation(es[:], sps[:], Exp, scale=SCALE)
                    if "av" in ablate:
                        continue
                    if pending is not None:
                        emit_av(*pending)
                    pending = (kt, es)
                for f in filler:
                    f()
                if "av" not in ablate and "exp" not in ablate:
                    emit_av(*pending)
                if "av" in ablate or "exp" in ablate:
                    return
                for j, av in ((0, av0), (1, av1)):
                    # custom-DVE recip requires matching in/out base
                    # partitions; den sits at PSUM partition 64, so hop it
                    # to partition 0 first (stock copy handles the shift).
                    den = nrm_pool.tile([1, 512], f32, tag="den")
                    nc.vector.tensor_copy(den[:], av[DH:DH + 1, :])
                    recip = nrm_pool.tile([1, 512], f32, tag="recip")
                    nc.vector.reciprocal_approx_fast(recip[:], den[:])
                    bcs = bcs_pool.tile([DH, 512], f32, tag="bcs")
                    nc.gpsimd.partition_broadcast(bcs[:], recip[:])
                    nc.vector.tensor_tensor(
                        outT[j * DH:(j + 1) * DH, qsl],
                        av[0:DH, :], bcs[:], mybir.AluOpType.mult)

            def emit_outproj_chunk(outTs, tt, ec):
                # all four pairs' contributions accumulated in one PSUM
                # group, then a single copy + write DMA per y region.
                # ps lives in psM (not psAV) so outproj matmuls never wait
                # on the softmax epilogue's reads of the av tiles.
                esl = slice(ec * 512, (ec + 1) * 512)
                ps = psM.tile([P, 512], f32, tag="m")
                for pr in range(MQ):
                    nc.tensor.matmul(ps[:], outTs[pr][:, tt * P:(tt + 1) * P],
                                     woT[:, pr, esl],
                                     start=(pr == 0), stop=(pr == MQ - 1))
                ysb = y_pool.tile([P, 512], f32, tag="ysb")
                nc.vector.tensor_copy(ysb[:], ps[:])
                dma = nc.sync.dma_start(y_ap[tt * P:(tt + 1) * P, esl], ysb[:])
                if (tt, ec) in y_prev_dma:
                    add_dep_helper(dma.ins, y_prev_dma[(tt, ec)].ins,
                                   reason="y write order across reps")
                y_prev_dma[(tt, ec)] = dma

            def drain(it, n):
                for _ in range(n):
                    args = next(it, None)
                    if args is None:
                        return
                    if len(args) == 4 and isinstance(args[0], int) and args[0] < MQ and not hasattr(args[1], "shape"):
                        emit_qk_group(*args)
                    else:
                        emit_outproj_chunk(*args)

            # software pipeline over head pairs:
            #   v | qk(0) | B(0)+qk(1)+C(0) | B(1)+qk(2)+C(1) | ... | B(3)+C(3)
            emit_v()
            qT, kT = alloc_qk(0)
            for gi, g in enumerate(qk_groups(0, qT, kT)):
                emit_qk_group(*g, rot=gi)
            outTs = []
            for pair in range(MQ):
                outT = outT_pool.tile([P, NT], bf16, tag="outT", name=f"outT{pair}")
                outTs.append(outT)
                if pair + 1 < MQ:
                    nqT, nkT = alloc_qk(pair + 1)
                    qk_iter = qk_groups(pair + 1, nqT, nkT)
                else:
                    nqT = nkT = None
                    qk_iter = iter(())
                for tq in range(TQ):
                    # filler: up to 6 queued out-projection chunks (queued
                    # at least one window ago, so their inputs are ready)
                    # plus this window's share of next-pair qk groups.
                    filler = []
                    for _ in range(min(6, len(op_queue))):
                        filler.append(op_queue.pop(0))
                    for _ in range(2):
                        g = next(qk_iter, None)
                        if g is not None:
                            filler.append(lambda g=g: emit_qk_group(*g))
                    if "scores" not in ablate:
                        emit_attn_tq(pair, qT, kT, outT, tq, filler)
                    else:
                        for f in filler:
                            f()
                    if "outproj" not in ablate and pair == MQ - 1:
                        # this tq's tokens are now complete across all four
                        # pairs: queue their out-projection chunks.
                        for tt in range(tq * 4, tq * 4 + 4):
                            for ec in range(E // 512):
                                op_queue.append(
                                    lambda o=list(outTs), tt=tt, ec=ec:
                                    emit_outproj_chunk(o, tt, ec))
                qT, kT = nqT, nkT

        if loop:
            with tc.For_i(0, rep, 1):
                emit_body()
        else:
            for _ in range(rep):
                emit_body()
        for f in op_queue:
            f()
        op_queue.clear()

    nc.compile()
    return nc


MMDT = "bf16"


def _get_nc(rep=1, ablate=(), mmdt=None):
    mmdt = mmdt or MMDT
    key = ("nc", rep, tuple(sorted(ablate)), mmdt)
    if key not in _cache:
        _cache[key] = _build(rep, ablate, mmdt)
    return _cache[key]


def make_in_maps(x, w_qkv, w_out, mmdt=None):
    import ml_dtypes
    mmdt = mmdt or MMDT
    dt = {"f32r": np.float32, "bf16": ml_dtypes.bfloat16,
          "fp16": np.float16, "mix": np.float16}[mmdt]
    wo_np = {"f32r": np.float32, "bf16": ml_dtypes.bfloat16,
             "fp16": np.float16, "mix": ml_dtypes.bfloat16}[mmdt]
    x = np.asarray(x, dtype=np.float32).astype(dt)
    w_qkv = np.asarray(w_qkv, dtype=np.float32).astype(dt)
    w_out = np.asarray(w_out, dtype=np.float32).astype(wo_np)
    in_maps = []
    for c in range(N_CORES):
        b, hh = divmod(c, 2)
        hsl = slice(hh * HD, (hh + 1) * HD)
        in_maps.append({
            "xT": np.ascontiguousarray(x[b].T),
            "wqT": np.ascontiguousarray(w_qkv[0 * E:1 * E][hsl].T),
            "wkT": np.ascontiguousarray(w_qkv[1 * E:2 * E][hsl].T),
            "wvT": np.ascontiguousarray(w_qkv[2 * E:3 * E][hsl].T),
            "woT": np.ascontiguousarray(w_out[:, hsl].T),
        })
    return in_maps


def combine_outputs(results):
    y = np.empty((B, NT, E), dtype=np.float32)
    for b in range(B):
        y[b] = results[2 * b]["y"] + results[2 * b + 1]["y"]
    return y


def kernel(x, w_qkv, w_out):
    from concourse.bass_utils import run_bass_kernel_spmd
    nc = _get_nc()
    in_maps = make_in_maps(x, w_qkv, w_out)
    res = run_bass_kernel_spmd(nc, in_maps, core_ids=list(range(N_CORES)))
    return combine_outputs(res.results)



# revision 32
# speedup vs baseline: 1.1626x; 1.0490x over previous
"""Multi-head attention (dense transformer block) for 8 Trainium2 NeuronCores.

Problem: x [4, 2048, 1024] f32, w_qkv [3072, 1024], w_out [1024, 1024]
  qkv = x @ w_qkv.T ; split q,k,v ; 16 heads x 64 dims
  out = softmax(q k^T / 8) v ; y = out @ w_out.T

Sharding: 8 shards = (batch b in 0..3) x (head-half hh in 0..1).
Each core handles one batch and 8 heads end-to-end: QKV projection
column-split, attention for its 8 heads, out-projection row-split ->
partial y. Host sums the two partial y's per batch. No collectives.

Kernel structure (engines run their instruction streams in order, so the
phases are emitted as a software pipeline over head pairs):

    qk(0) | v | B(0) qk(1) C(0) | B(1) qk(2) C(1) | ... | B(3) C(3)

  - qk(p): q^T,k^T [128, tok] for pair p (fp32r matmuls, rotating bufs)
  - v: value projection -> vaug bf16 [ktok, head, 65] with a ones column
  - B(p): attention. Scores computed transposed per head S^T[ktok, qtok]
    with the two heads PAIRED via PE row-tiling (K=64 at partitions
    0/64) into adjacent PSUM banks; one ScalarE exp ACTIVATE [128, 1024]
    per k-tile covers both heads with the 1/8 scale folded in (softmax
    max-subtraction skipped; scores are O(+-6)). AV matmuls in bf16 with
    M=65: the ones column makes PSUM row 64 the softmax denominators.
    Normalization: DVE reciprocal -> GpSimd partition-broadcast -> DVE
    multiply (PE stays out of the softmax epilogue).
  - C(p): per-pair out-projection (K=128), accumulated into y in DRAM
    (first pair writes, later pairs DMA-accumulate).
"""

import numpy as np

B = 4
NT = 2048          # tokens per batch
E = 1024           # embed dim
H = 16             # heads
DH = 64            # head dim
HD = 512           # head dims per core (8 heads)
N_CORES = 8
SCALE = DH ** -0.5
P = 128

# DVE Schraudolph fast-exp: bf16 bits of exp(SCALE*s) ~= int16(s*FE_S + FE_B)
# (bf16 = 8-bit exponent + 7-bit mantissa; linear-mantissa approx, +-3% rel
# err, bias cancels in softmax). Lets the DVE take half the softmax exps.
import math
FE_S = SCALE * 128.0 / math.log(2.0)
FE_B = 127.0 * 128.0 - 5.5

_cache = {}


def _build(rep=1, ablate=(), mmdt="f32r", loop=False):
    import concourse.mybir as mybir
    import concourse.tile as tile
    from concourse import bacc
    from contextlib import ExitStack

    # dtype scheme: f32r/bf16/fp16 uniform; "mix" = fp16 q/k path + bf16 soft path
    f32 = mybir.dt.float32
    _qk = {"f32r": mybir.dt.float32r, "bf16": mybir.dt.bfloat16,
           "fp16": mybir.dt.float16, "mix": mybir.dt.float16}
    _soft = {"f32r": mybir.dt.bfloat16, "bf16": mybir.dt.bfloat16,
             "fp16": mybir.dt.float16, "mix": mybir.dt.bfloat16}
    f32r = _qk[mmdt]          # q/k-side matmul dtype (x, wq, wk, wv, qT, kT)
    bf16 = _soft[mmdt]        # softmax/out-side dtype (es, vaug, outT, woT)
    in_dt = {"f32r": f32, "bf16": mybir.dt.bfloat16,
             "fp16": mybir.dt.float16, "mix": mybir.dt.float16}[mmdt]
    wo_dt = {"f32r": f32, "bf16": mybir.dt.bfloat16,
             "fp16": mybir.dt.float16, "mix": mybir.dt.bfloat16}[mmdt]
    Exp = mybir.ActivationFunctionType.Exp
    Add = mybir.AluOpType.add

    nc = bacc.Bacc("TRN2", target_bir_lowering=False, debug=False,
                   enable_asserts=False, num_devices=N_CORES)

    xT_ap = nc.dram_tensor("xT", [E, NT], in_dt, kind="ExternalInput").ap()
    wqT_ap = nc.dram_tensor("wqT", [E, HD], in_dt, kind="ExternalInput").ap()
    wkT_ap = nc.dram_tensor("wkT", [E, HD], in_dt, kind="ExternalInput").ap()
    wvT_ap = nc.dram_tensor("wvT", [E, HD], in_dt, kind="ExternalInput").ap()
    woT_ap = nc.dram_tensor("woT", [HD, E], wo_dt, kind="ExternalInput").ap()
    y_ap = nc.dram_tensor("y", [NT, E], f32, kind="ExternalOutput").ap()

    KE = E // P        # 8 contraction tiles over embed
    MQ = HD // P       # 4 partition tiles over head dims = head pairs
    TQ = NT // 512     # 4 query chunks of 512
    TT = NT // P       # 16 token tiles of 128

    from concourse.tile_rust import add_dep_helper

    with tile.TileContext(nc) as tc, ExitStack() as ctx:
        per = ctx.enter_context(tc.tile_pool(name="per", bufs=1))
        qk_pool = ctx.enter_context(tc.tile_pool(name="qk", bufs=2))
        outT_pool = ctx.enter_context(tc.tile_pool(name="ot", bufs=5))
        es_pool = ctx.enter_context(tc.tile_pool(name="es", bufs=3))
        y_pool = ctx.enter_context(tc.tile_pool(name="ysb", bufs=2))
        nrm_pool = ctx.enter_context(tc.tile_pool(name="nrm", bufs=2))
        bcs_pool = ctx.enter_context(tc.tile_pool(name="bcs", bufs=2))
        xT_pool = ctx.enter_context(tc.tile_pool(name="xTp", bufs=2))
        psS = ctx.enter_context(tc.tile_pool(name="psS", bufs=2, space="PSUM"))
        psAV = ctx.enter_context(tc.tile_pool(name="psAV", bufs=2, space="PSUM"))
        psM = ctx.enter_context(tc.tile_pool(name="psM", bufs=2, space="PSUM"))

        # rep-invariant weights (wv first: the value projection runs first)
        wv = per.tile([P, KE, HD], f32r, tag="wv")
        nc.scalar.dma_start(wv[:], wvT_ap.rearrange("(o p) m -> p o m", p=P).bitcast(f32r))
        wq = per.tile([P, KE, HD], f32r, tag="wq")
        nc.scalar.dma_start(wq[:], wqT_ap.rearrange("(o p) m -> p o m", p=P).bitcast(f32r))
        wk = per.tile([P, KE, HD], f32r, tag="wk")
        nc.scalar.dma_start(wk[:], wkT_ap.rearrange("(o p) m -> p o m", p=P).bitcast(f32r))
        woT = per.tile([P, MQ, E], bf16, tag="woT")
        nc.scalar.dma_start(woT[:], woT_ap.rearrange("(o p) e -> p o e", p=P).bitcast(bf16))
        # double-buffered value tiles: rep r uses parity r % 2 so the next
        # rep's value projection can run as filler inside this rep.
        vaug_sets = []
        for par in range(2):
            vg = [per.tile([P, 4, 8, DH + 1], bf16, tag=f"vaug{par}_{g}",
                           name=f"vaug{par}_{g}") for g in range(TT // 4)]
            for g in range(TT // 4):
                nc.vector.memset(vg[g][:, :, :, DH:DH + 1], 1.0)
            vaug_sets.append([vg[t // 4][:, t % 4] for t in range(TT)])

        # Tile does not order DMAs by DRAM range: chain each y region's
        # write DMAs explicitly across reps.
        y_prev_dma = {}
        # deferred work queue: (pe_weight, closure) for out-projection
        # chunks and next-rep value-projection groups. Items drain as
        # kt-loop filler across pair and rep boundaries so this work rides
        # in the exp-gated PE bubbles instead of forming serial phases.
        work_queue = []

        def pop_filler(budget):
            items = []
            while work_queue and budget > 0:
                w, c = work_queue[0]
                if w > budget and items:
                    break
                work_queue.pop(0)
                items.append(c)
                budget -= w
            return items

        xT_src = xT_ap.rearrange("(o p) t -> p o t", p=P).bitcast(f32r)

        def emit_xT_dmas(gen):
            # on the GpSimd DMA queue: the sync queue carries the y writes
            # and an xT load would head-block them for ~3us each.
            xTs = []
            for ke in range(KE):
                xk = xT_pool.tile([P, NT], f32r, tag=f"xT{ke}",
                                  name=f"xT{ke}g{gen}")
                nc.gpsimd.dma_start(xk[:], xT_src[:, ke, :])
                xTs.append(xk)
            return xTs

        def emit_qk_group(xTs, mq, dst, w, tq, rot=0):
            ps = psM.tile([P, 512], f32, tag="m")
            for i in range(KE):
                ke = (i + rot) % KE
                nc.tensor.matmul(ps[:], w[:, ke, mq * P:(mq + 1) * P],
                                 xTs[ke][:, tq * 512:(tq + 1) * 512],
                                 start=(i == 0), stop=(i == KE - 1))
            nc.vector.tensor_copy(dst[:, tq * 512:(tq + 1) * 512], ps[:])

        def alloc_qk(mq):
            qT = qk_pool.tile([P, NT], f32r, tag="qTp", name=f"qT{mq}")
            kT = qk_pool.tile([P, NT], f32r, tag="kTp", name=f"kT{mq}")
            return qT, kT

        def qk_groups(mq, qT, kT):
            for dst, w in ((kT, wk), (qT, wq)):
                for tq in range(TQ):
                    yield (mq, dst, w, tq)

        def emit_v_group(xTs, vaugs, tt):
            ps = psM.tile([P, HD], f32, tag="m")
            for i in range(KE):
                ke = (i + tt) % KE
                nc.tensor.matmul(ps[:], xTs[ke][:, tt * P:(tt + 1) * P],
                                 wv[:, ke, :], start=(i == 0), stop=(i == KE - 1))
            nc.vector.tensor_copy(vaugs[tt][:, :, 0:DH],
                                  ps[:].rearrange("p (h d) -> p h d", h=8))

        def emit_attn_tq(vaugs, pair, qT, kT, outT, tq, filler=()):
            filler = list(filler)
            qsl = slice(tq * 512, (tq + 1) * 512)
            av0 = psAV.tile([DH + 1, 512], f32, tag="av")
            av1 = psAV.tile([DH + 1, 512], f32, tag="av")

            def emit_av(kt, es):
                nc.tensor.matmul(av0[:], vaugs[kt][:, 2 * pair, :], es[:, 0, :],
                                 start=(kt == 0), stop=(kt == TT - 1))
                nc.tensor.matmul(av1[:], vaugs[kt][:, 2 * pair + 1, :], es[:, 1, :],
                                 start=(kt == 0), stop=(kt == TT - 1))

            # AV lags scores/exp by one k-tile so the PE never sits in
            # the scores -> exp -> AV semaphore chain: while ScalarE
            # exps tile kt, the PE already runs scores of kt+1.
            pending = None
            for kt in range(TT):
                # drain filler work spread across the kt loop: the
                # exp-gated PE bubbles absorb the matmuls and the DVE
                # copies interleave with the loop instead of ganging up
                # at tq boundaries.
                if filler:
                    for _ in range(-(-len(filler) // (TT - kt))):
                        filler.pop(0)()
                ksl = slice(kt * P, (kt + 1) * P)
                sps = psS.tile([P, 2, 512], f32, tag="s")
                nc.tensor.matmul(sps[:, 0, :], kT[0:DH, ksl],
                                 qT[0:DH, qsl], start=True, stop=True)
                nc.tensor.matmul(sps[:, 1, :], kT[DH:P, ksl],
                                 qT[DH:P, qsl], start=True, stop=True)
                if "exp" in ablate:
                    continue
                es = es_pool.tile([P, 2, 512], bf16, tag="es")
                nc.scalar.activation(es[:], sps[:], Exp, scale=SCALE)
                if "av" in ablate:
                    continue
                if pending is not None:
                    emit_av(*pending)
                pending = (kt, es)
            for f in filler:
                f()
            if "av" in ablate or "exp" in ablate:
                return
            emit_av(*pending)
            for j, av in ((0, av0), (1, av1)):
                # custom-DVE recip requires matching in/out base
                # partitions; den sits at PSUM partition 64, so hop it
                # to partition 0 first (stock copy handles the shift).
                den = nrm_pool.tile([1, 512], f32, tag="den")
                nc.vector.tensor_copy(den[:], av[DH:DH + 1, :])
                recip = nrm_pool.tile([1, 512], f32, tag="recip")
                nc.vector.reciprocal_approx_fast(recip[:], den[:])
                bcs = bcs_pool.tile([DH, 512], f32, tag="bcs")
                nc.gpsimd.partition_broadcast(bcs[:], recip[:])
                nc.vector.tensor_tensor(
                    outT[j * DH:(j + 1) * DH, qsl],
                    av[0:DH, :], bcs[:], mybir.AluOpType.mult)

        def emit_outproj_chunk(outTs, tt, ec):
            # all four pairs' contributions accumulated in one PSUM
            # group, then a single copy + write DMA per y region.
            # ps lives in psM (not psAV) so outproj matmuls never wait
            # on the softmax epilogue's reads of the av tiles.
            esl = slice(ec * 512, (ec + 1) * 512)
            ps = psM.tile([P, 512], f32, tag="m")
            for pr in range(MQ):
                nc.tensor.matmul(ps[:], outTs[pr][:, tt * P:(tt + 1) * P],
                                 woT[:, pr, esl],
                                 start=(pr == 0), stop=(pr == MQ - 1))
            ysb = y_pool.tile([P, 512], f32, tag="ysb")
            nc.vector.tensor_copy(ysb[:], ps[:])
            dma = nc.sync.dma_start(y_ap[tt * P:(tt + 1) * P, esl], ysb[:])
            if (tt, ec) in y_prev_dma:
                add_dep_helper(dma.ins, y_prev_dma[(tt, ec)].ins,
                               reason="y write order across reps")
            y_prev_dma[(tt, ec)] = dma

        def emit_body(pre, nxt_gen):
            """One rep: attention pairs 0-3 using tiles prepared by the
            previous rep's pipeline, while preparing the next rep's
            inputs (xT DMAs at pair 0, value groups queued at pair 2,
            qk(0) as pair 3's direct filler)."""
            xTs, vaugs, qT, kT = pre
            nxt_xTs = nxt_vaugs = nxt_qT = nxt_kT = None
            outTs = []
            for pair in range(MQ):
                outT = outT_pool.tile([P, NT], bf16, tag="outT", name=f"outT{pair}")
                outTs.append(outT)
                if pair == 0 and nxt_gen is not None:
                    nxt_xTs = emit_xT_dmas(nxt_gen)
                    nxt_vaugs = vaug_sets[nxt_gen % 2]
                if pair == 2 and nxt_gen is not None:
                    for tt in range(TT):
                        work_queue.append(
                            (2, lambda x=nxt_xTs, v=nxt_vaugs, tt=tt:
                             emit_v_group(x, v, tt)))
                if pair + 1 < MQ:
                    nqT, nkT = alloc_qk(pair + 1)
                    qk_iter = qk_groups(pair + 1, nqT, nkT)
                    qk_xTs = xTs
                elif nxt_gen is not None:
                    # pair 3's direct filler is the NEXT rep's qk(0)
                    nxt_qT, nxt_kT = alloc_qk(0)
                    qk_iter = qk_groups(0, nxt_qT, nxt_kT)
                    qk_xTs = nxt_xTs
                else:
                    nqT = nkT = None
                    qk_iter = iter(())
                    qk_xTs = xTs
                for tq in range(TQ):
                    filler = pop_filler(6)
                    for _ in range(2):
                        g = next(qk_iter, None)
                        if g is not None:
                            filler.append(
                                lambda g=g, x=qk_xTs: emit_qk_group(x, *g))
                    if "scores" not in ablate:
                        emit_attn_tq(vaugs, pair, qT, kT, outT, tq, filler)
                    else:
                        for f in filler:
                            f()
                    if "outproj" not in ablate and pair == MQ - 1:
                        # this tq's tokens are now complete across all four
                        # pairs: queue their out-projection chunks.
                        for tt in range(tq * 4, tq * 4 + 4):
                            for ec in range(E // 512):
                                work_queue.append(
                                    (1, lambda o=list(outTs), tt=tt, ec=ec:
                                     emit_outproj_chunk(o, tt, ec)))
                if pair + 1 < MQ:
                    qT, kT = nqT, nkT
            return (nxt_xTs, nxt_vaugs, nxt_qT, nxt_kT)

        def emit_prologue(gen):
            # unpipelined lead-in for the first rep (and the loop path)
            xTs = emit_xT_dmas(gen)
            vaugs = vaug_sets[gen % 2]
            for tt in range(TT):
                emit_v_group(xTs, vaugs, tt)
            qT, kT = alloc_qk(0)
            for gi, g in enumerate(qk_groups(0, qT, kT)):
                emit_qk_group(xTs, *g, rot=gi)
            return (xTs, vaugs, qT, kT)

        def drain_queue():
            for w, f in work_queue:
                f()
            work_queue.clear()

        if loop:
            with tc.For_i(0, rep, 1):
                pre = emit_prologue(0)
                emit_body(pre, None)
                drain_queue()
        else:
            pre = emit_prologue(0)
            for r in range(rep):
                pre = emit_body(pre, r + 1 if r + 1 < rep else None)
            drain_queue()

    nc.compile()
    return nc


MMDT = "bf16"


def _get_nc(rep=1, ablate=(), mmdt=None):
    mmdt = mmdt or MMDT
    key = ("nc", rep, tuple(sorted(ablate)), mmdt)
    if key not in _cache:
        _cache[key] = _build(rep, ablate, mmdt)
    return _cache[key]


def make_in_maps(x, w_qkv, w_out, mmdt=None):
    import ml_dtypes
    mmdt = mmdt or MMDT
    dt = {"f32r": np.float32, "bf16": ml_dtypes.bfloat16,
          "fp16": np.float16, "mix": np.float16}[mmdt]
    wo_np = {"f32r": np.float32, "bf16": ml_dtypes.bfloat16,
             "fp16": np.float16, "mix": ml_dtypes.bfloat16}[mmdt]
    x = np.asarray(x, dtype=np.float32).astype(dt)
    w_qkv = np.asarray(w_qkv, dtype=np.float32).astype(dt)
    w_out = np.asarray(w_out, dtype=np.float32).astype(wo_np)
    in_maps = []
    for c in range(N_CORES):
        b, hh = divmod(c, 2)
        hsl = slice(hh * HD, (hh + 1) * HD)
        in_maps.append({
            "xT": np.ascontiguousarray(x[b].T),
            "wqT": np.ascontiguousarray(w_qkv[0 * E:1 * E][hsl].T),
            "wkT": np.ascontiguousarray(w_qkv[1 * E:2 * E][hsl].T),
            "wvT": np.ascontiguousarray(w_qkv[2 * E:3 * E][hsl].T),
            "woT": np.ascontiguousarray(w_out[:, hsl].T),
        })
    return in_maps


def combine_outputs(results):
    y = np.empty((B, NT, E), dtype=np.float32)
    for b in range(B):
        y[b] = results[2 * b]["y"] + results[2 * b + 1]["y"]
    return y


def kernel(x, w_qkv, w_out):
    from concourse.bass_utils import run_bass_kernel_spmd
    nc = _get_nc()
    in_maps = make_in_maps(x, w_qkv, w_out)
    res = run_bass_kernel_spmd(nc, in_maps, core_ids=list(range(N_CORES)))
    return combine_outputs(res.results)



# revision 35
# speedup vs baseline: 1.2255x; 1.0541x over previous
"""Multi-head attention (dense transformer block) for 8 Trainium2 NeuronCores.

Problem: x [4, 2048, 1024] f32, w_qkv [3072, 1024], w_out [1024, 1024]
  qkv = x @ w_qkv.T ; split q,k,v ; 16 heads x 64 dims
  out = softmax(q k^T / 8) v ; y = out @ w_out.T

Sharding: 8 shards = (batch b in 0..3) x (head-half hh in 0..1).
Each core handles one batch and 8 heads end-to-end: QKV projection
column-split, attention for its 8 heads, out-projection row-split ->
partial y. Host sums the two partial y's per batch. No collectives.

Kernel structure (engines run their instruction streams in order, so the
phases are emitted as a software pipeline over head pairs):

    qk(0) | v | B(0) qk(1) C(0) | B(1) qk(2) C(1) | ... | B(3) C(3)

  - qk(p): q^T,k^T [128, tok] for pair p (fp32r matmuls, rotating bufs)
  - v: value projection -> vaug bf16 [ktok, head, 65] with a ones column
  - B(p): attention. Scores computed transposed per head S^T[ktok, qtok]
    with the two heads PAIRED via PE row-tiling (K=64 at partitions
    0/64) into adjacent PSUM banks; one ScalarE exp ACTIVATE [128, 1024]
    per k-tile covers both heads with the 1/8 scale folded in (softmax
    max-subtraction skipped; scores are O(+-6)). AV matmuls in bf16 with
    M=65: the ones column makes PSUM row 64 the softmax denominators.
    Normalization: DVE reciprocal -> GpSimd partition-broadcast -> DVE
    multiply (PE stays out of the softmax epilogue).
  - C(p): per-pair out-projection (K=128), accumulated into y in DRAM
    (first pair writes, later pairs DMA-accumulate).
"""

import numpy as np

B = 4
NT = 2048          # tokens per batch
E = 1024           # embed dim
H = 16             # heads
DH = 64            # head dim
HD = 512           # head dims per core (8 heads)
N_CORES = 8
SCALE = DH ** -0.5
P = 128

# DVE Schraudolph fast-exp: bf16 bits of exp(SCALE*s) ~= int16(s*FE_S + FE_B)
# (bf16 = 8-bit exponent + 7-bit mantissa; linear-mantissa approx, +-3% rel
# err, bias cancels in softmax). Lets the DVE take half the softmax exps.
import math
FE_S = SCALE * 128.0 / math.log(2.0)
FE_B = 127.0 * 128.0 - 5.5

_cache = {}


def _build(rep=1, ablate=(), mmdt="f32r", loop=False):
    import concourse.mybir as mybir
    import concourse.tile as tile
    from concourse import bacc
    from contextlib import ExitStack

    # dtype scheme: f32r/bf16/fp16 uniform; "mix" = fp16 q/k path + bf16 soft path
    f32 = mybir.dt.float32
    _qk = {"f32r": mybir.dt.float32r, "bf16": mybir.dt.bfloat16,
           "fp16": mybir.dt.float16, "mix": mybir.dt.float16}
    _soft = {"f32r": mybir.dt.bfloat16, "bf16": mybir.dt.bfloat16,
             "fp16": mybir.dt.float16, "mix": mybir.dt.bfloat16}
    f32r = _qk[mmdt]          # q/k-side matmul dtype (x, wq, wk, wv, qT, kT)
    bf16 = _soft[mmdt]        # softmax/out-side dtype (es, vaug, outT, woT)
    in_dt = {"f32r": f32, "bf16": mybir.dt.bfloat16,
             "fp16": mybir.dt.float16, "mix": mybir.dt.float16}[mmdt]
    wo_dt = {"f32r": f32, "bf16": mybir.dt.bfloat16,
             "fp16": mybir.dt.float16, "mix": mybir.dt.bfloat16}[mmdt]
    Exp = mybir.ActivationFunctionType.Exp
    Add = mybir.AluOpType.add

    nc = bacc.Bacc("TRN2", target_bir_lowering=False, debug=False,
                   enable_asserts=False, num_devices=N_CORES)

    xT_ap = nc.dram_tensor("xT", [E, NT], in_dt, kind="ExternalInput").ap()
    wqT_ap = nc.dram_tensor("wqT", [E, HD], in_dt, kind="ExternalInput").ap()
    wkT_ap = nc.dram_tensor("wkT", [E, HD], in_dt, kind="ExternalInput").ap()
    wvT_ap = nc.dram_tensor("wvT", [E, HD], in_dt, kind="ExternalInput").ap()
    woT_ap = nc.dram_tensor("woT", [HD, E], wo_dt, kind="ExternalInput").ap()
    y_ap = nc.dram_tensor("y", [NT, E], f32, kind="ExternalOutput").ap()

    KE = E // P        # 8 contraction tiles over embed
    MQ = HD // P       # 4 partition tiles over head dims = head pairs
    TQ = NT // 512     # 4 query chunks of 512
    TT = NT // P       # 16 token tiles of 128

    from concourse.tile_rust import add_dep_helper

    with tile.TileContext(nc) as tc, ExitStack() as ctx:
        per = ctx.enter_context(tc.tile_pool(name="per", bufs=1))
        qk_pool = ctx.enter_context(tc.tile_pool(name="qk", bufs=2))
        outT_pool = ctx.enter_context(tc.tile_pool(name="ot", bufs=5))
        es_pool = ctx.enter_context(tc.tile_pool(name="es", bufs=3))
        y_pool = ctx.enter_context(tc.tile_pool(name="ysb", bufs=2))
        nrm_pool = ctx.enter_context(tc.tile_pool(name="nrm", bufs=2))
        bcs_pool = ctx.enter_context(tc.tile_pool(name="bcs", bufs=2))
        xT_pool = ctx.enter_context(tc.tile_pool(name="xTp", bufs=2))
        psS = ctx.enter_context(tc.tile_pool(name="psS", bufs=2, space="PSUM"))
        psAV = ctx.enter_context(tc.tile_pool(name="psAV", bufs=2, space="PSUM"))
        psM = ctx.enter_context(tc.tile_pool(name="psM", bufs=2, space="PSUM"))

        # rep-invariant weights (wv first: the value projection runs first)
        wv = per.tile([P, KE, HD], f32r, tag="wv")
        nc.scalar.dma_start(wv[:], wvT_ap.rearrange("(o p) m -> p o m", p=P).bitcast(f32r))
        wq = per.tile([P, KE, HD], f32r, tag="wq")
        nc.scalar.dma_start(wq[:], wqT_ap.rearrange("(o p) m -> p o m", p=P).bitcast(f32r))
        wk = per.tile([P, KE, HD], f32r, tag="wk")
        nc.scalar.dma_start(wk[:], wkT_ap.rearrange("(o p) m -> p o m", p=P).bitcast(f32r))
        woT = per.tile([P, MQ, E], bf16, tag="woT")
        nc.scalar.dma_start(woT[:], woT_ap.rearrange("(o p) e -> p o e", p=P).bitcast(bf16))
        # double-buffered value tiles: rep r uses parity r % 2 so the next
        # rep's value projection can run as filler inside this rep.
        # Each head's slice is read by the AV matmul through a 128-column
        # window (65 real columns + overrun into the next head; 63 pad
        # columns after the last head) so every LDWEIGHTS is a full
        # 128-column load — that enables FWL + background-buffer overlap,
        # hiding the weight load under the previous AV matmul.
        VW = 8 * (DH + 1)          # 520 real columns per token row
        vaug_sets = []
        for par in range(2):
            vg = [per.tile([P, 4, VW + 63], bf16, tag=f"vaug{par}_{g}",
                           name=f"vaug{par}_{g}") for g in range(TT // 4)]
            for g in range(TT // 4):
                hv = vg[g][:, :, 0:VW].rearrange("p a (h c) -> p a h c", h=8)
                nc.vector.memset(hv[:, :, :, DH:DH + 1], 1.0)
            vaug_sets.append([vg[t // 4][:, t % 4] for t in range(TT)])

        # Tile does not order DMAs by DRAM range: chain each y region's
        # write DMAs explicitly across reps.
        y_prev_dma = {}
        # deferred work queue: (pe_weight, closure) for out-projection
        # chunks and next-rep value-projection groups. Items drain as
        # kt-loop filler across pair and rep boundaries so this work rides
        # in the exp-gated PE bubbles instead of forming serial phases.
        work_queue = []

        def pop_filler(budget):
            items = []
            while work_queue and budget > 0:
                w, c = work_queue[0]
                if w > budget and items:
                    break
                work_queue.pop(0)
                items.append(c)
                budget -= w
            return items

        xT_src = xT_ap.rearrange("(o p) t -> p o t", p=P).bitcast(f32r)

        def emit_xT_dmas(gen):
            # on the GpSimd DMA queue: the sync queue carries the y writes
            # and an xT load would head-block them for ~3us each.
            xTs = []
            for ke in range(KE):
                xk = xT_pool.tile([P, NT], f32r, tag=f"xT{ke}",
                                  name=f"xT{ke}g{gen}")
                nc.gpsimd.dma_start(xk[:], xT_src[:, ke, :])
                xTs.append(xk)
            return xTs

        def emit_qk_group(xTs, mq, dst, w, tq, rot=0):
            ps = psM.tile([P, 512], f32, tag="m")
            for i in range(KE):
                ke = (i + rot) % KE
                nc.tensor.matmul(ps[:], w[:, ke, mq * P:(mq + 1) * P],
                                 xTs[ke][:, tq * 512:(tq + 1) * 512],
                                 start=(i == 0), stop=(i == KE - 1))
            nc.vector.tensor_copy(dst[:, tq * 512:(tq + 1) * 512], ps[:])

        def alloc_qk(mq):
            qT = qk_pool.tile([P, NT], f32r, tag="qTp", name=f"qT{mq}")
            kT = qk_pool.tile([P, NT], f32r, tag="kTp", name=f"kT{mq}")
            return qT, kT

        def qk_groups(mq, qT, kT):
            for dst, w in ((kT, wk), (qT, wq)):
                for tq in range(TQ):
                    yield (mq, dst, w, tq)

        def emit_v_group(xTs, vaugs, tt):
            ps = psM.tile([P, HD], f32, tag="m")
            for i in range(KE):
                ke = (i + tt) % KE
                nc.tensor.matmul(ps[:], xTs[ke][:, tt * P:(tt + 1) * P],
                                 wv[:, ke, :], start=(i == 0), stop=(i == KE - 1))
            dst = vaugs[tt][:, 0:VW].rearrange("p (h c) -> p h c", h=8)
            nc.vector.tensor_copy(dst[:, :, 0:DH],
                                  ps[:].rearrange("p (h d) -> p h d", h=8))

        def emit_attn_tq(vaugs, pair, qT, kT, outT, tq, filler=()):
            filler = list(filler)
            qsl = slice(tq * 512, (tq + 1) * 512)
            av0 = psAV.tile([P, 512], f32, tag="av")
            av1 = psAV.tile([P, 512], f32, tag="av")

            def emit_av(kt, es):
                # stationary = 128-column window starting at this head's
                # slice: full-width LDWEIGHTS (FWL + background buffer).
                # PSUM rows 65-127 accumulate neighbor-head garbage that
                # the epilogue never reads.
                h0, h1 = 2 * pair, 2 * pair + 1
                nc.tensor.matmul(
                    av0[:], vaugs[kt][:, h0 * (DH + 1):h0 * (DH + 1) + P],
                    es[:, 0, :], start=(kt == 0), stop=(kt == TT - 1))
                nc.tensor.matmul(
                    av1[:], vaugs[kt][:, h1 * (DH + 1):h1 * (DH + 1) + P],
                    es[:, 1, :], start=(kt == 0), stop=(kt == TT - 1))

            # AV lags scores/exp by one k-tile so the PE never sits in
            # the scores -> exp -> AV semaphore chain: while ScalarE
            # exps tile kt, the PE already runs scores of kt+1.
            pending = None
            for kt in range(TT):
                # drain filler work spread across the kt loop: the
                # exp-gated PE bubbles absorb the matmuls and the DVE
                # copies interleave with the loop instead of ganging up
                # at tq boundaries.
                if filler:
                    for _ in range(-(-len(filler) // (TT - kt))):
                        filler.pop(0)()
                ksl = slice(kt * P, (kt + 1) * P)
                sps = psS.tile([P, 2, 512], f32, tag="s")
                nc.tensor.matmul(sps[:, 0, :], kT[0:DH, ksl],
                                 qT[0:DH, qsl], start=True, stop=True)
                nc.tensor.matmul(sps[:, 1, :], kT[DH:P, ksl],
                                 qT[DH:P, qsl], start=True, stop=True)
                if "exp" in ablate:
                    continue
                es = es_pool.tile([P, 2, 512], bf16, tag="es")
                nc.scalar.activation(es[:], sps[:], Exp, scale=SCALE)
                if "av" in ablate:
                    continue
                if pending is not None:
                    emit_av(*pending)
                pending = (kt, es)
            for f in filler:
                f()
            if "av" in ablate or "exp" in ablate:
                return
            emit_av(*pending)
            for j, av in ((0, av0), (1, av1)):
                # custom-DVE recip requires matching in/out base
                # partitions; den sits at PSUM partition 64, so hop it
                # to partition 0 first (stock copy handles the shift).
                den = nrm_pool.tile([1, 512], f32, tag="den")
                nc.vector.tensor_copy(den[:], av[DH:DH + 1, :])
                recip = nrm_pool.tile([1, 512], f32, tag="recip")
                nc.vector.reciprocal_approx_fast(recip[:], den[:])
                bcs = bcs_pool.tile([DH, 512], f32, tag="bcs")
                nc.gpsimd.partition_broadcast(bcs[:], recip[:])
                nc.vector.tensor_tensor(
                    outT[j * DH:(j + 1) * DH, qsl],
                    av[0:DH, :], bcs[:], mybir.AluOpType.mult)

        def emit_outproj_chunk(outTs, tt, ec):
            # all four pairs' contributions accumulated in one PSUM
            # group, then a single copy + write DMA per y region.
            # ps lives in psM (not psAV) so outproj matmuls never wait
            # on the softmax epilogue's reads of the av tiles.
            esl = slice(ec * 512, (ec + 1) * 512)
            ps = psM.tile([P, 512], f32, tag="m")
            for pr in range(MQ):
                nc.tensor.matmul(ps[:], outTs[pr][:, tt * P:(tt + 1) * P],
                                 woT[:, pr, esl],
                                 start=(pr == 0), stop=(pr == MQ - 1))
            ysb = y_pool.tile([P, 512], f32, tag="ysb")
            nc.vector.tensor_copy(ysb[:], ps[:])
            dma = nc.sync.dma_start(y_ap[tt * P:(tt + 1) * P, esl], ysb[:])
            if (tt, ec) in y_prev_dma:
                add_dep_helper(dma.ins, y_prev_dma[(tt, ec)].ins,
                               reason="y write order across reps")
            y_prev_dma[(tt, ec)] = dma

        def emit_body(pre, nxt_gen):
            """One rep: attention pairs 0-3 using tiles prepared by the
            previous rep's pipeline, while preparing the next rep's
            inputs (xT DMAs at pair 0, value groups queued at pair 2,
            qk(0) as pair 3's direct filler)."""
            xTs, vaugs, qT, kT = pre
            nxt_xTs = nxt_vaugs = nxt_qT = nxt_kT = None
            outTs = []
            for pair in range(MQ):
                outT = outT_pool.tile([P, NT], bf16, tag="outT", name=f"outT{pair}")
                outTs.append(outT)
                if pair == 0 and nxt_gen is not None:
                    nxt_xTs = emit_xT_dmas(nxt_gen)
                    nxt_vaugs = vaug_sets[nxt_gen % 2]
                if pair == 2 and nxt_gen is not None:
                    for tt in range(TT):
                        work_queue.append(
                            (2, lambda x=nxt_xTs, v=nxt_vaugs, tt=tt:
                             emit_v_group(x, v, tt)))
                if pair + 1 < MQ:
                    nqT, nkT = alloc_qk(pair + 1)
                    qk_iter = qk_groups(pair + 1, nqT, nkT)
                    qk_xTs = xTs
                elif nxt_gen is not None:
                    # pair 3's direct filler is the NEXT rep's qk(0)
                    nxt_qT, nxt_kT = alloc_qk(0)
                    qk_iter = qk_groups(0, nxt_qT, nxt_kT)
                    qk_xTs = nxt_xTs
                else:
                    nqT = nkT = None
                    qk_iter = iter(())
                    qk_xTs = xTs
                for tq in range(TQ):
                    filler = pop_filler(6)
                    for _ in range(2):
                        g = next(qk_iter, None)
                        if g is not None:
                            filler.append(
                                lambda g=g, x=qk_xTs: emit_qk_group(x, *g))
                    if "scores" not in ablate:
                        emit_attn_tq(vaugs, pair, qT, kT, outT, tq, filler)
                    else:
                        for f in filler:
                            f()
                    if "outproj" not in ablate and pair == MQ - 1:
                        # this tq's tokens are now complete across all four
                        # pairs: queue their out-projection chunks.
                        for tt in range(tq * 4, tq * 4 + 4):
                            for ec in range(E // 512):
                                work_queue.append(
                                    (1, lambda o=list(outTs), tt=tt, ec=ec:
                                     emit_outproj_chunk(o, tt, ec)))
                if pair + 1 < MQ:
                    qT, kT = nqT, nkT
            return (nxt_xTs, nxt_vaugs, nxt_qT, nxt_kT)

        def emit_prologue(gen):
            # unpipelined lead-in for the first rep (and the loop path)
            xTs = emit_xT_dmas(gen)
            vaugs = vaug_sets[gen % 2]
            for tt in range(TT):
                emit_v_group(xTs, vaugs, tt)
            qT, kT = alloc_qk(0)
            for gi, g in enumerate(qk_groups(0, qT, kT)):
                emit_qk_group(xTs, *g, rot=gi)
            return (xTs, vaugs, qT, kT)

        def drain_queue():
            for w, f in work_queue:
                f()
            work_queue.clear()

        if loop:
            with tc.For_i(0, rep, 1):
                pre = emit_prologue(0)
                emit_body(pre, None)
                drain_queue()
        else:
            pre = emit_prologue(0)
            for r in range(rep):
                pre = emit_body(pre, r + 1 if r + 1 < rep else None)
            drain_queue()

    nc.compile()
    return nc


MMDT = "bf16"


def _get_nc(rep=1, ablate=(), mmdt=None):
    mmdt = mmdt or MMDT
    key = ("nc", rep, tuple(sorted(ablate)), mmdt)
    if key not in _cache:
        _cache[key] = _build(rep, ablate, mmdt)
    return _cache[key]


def make_in_maps(x, w_qkv, w_out, mmdt=None):
    import ml_dtypes
    mmdt = mmdt or MMDT
    dt = {"f32r": np.float32, "bf16": ml_dtypes.bfloat16,
          "fp16": np.float16, "mix": np.float16}[mmdt]
    wo_np = {"f32r": np.float32, "bf16": ml_dtypes.bfloat16,
             "fp16": np.float16, "mix": ml_dtypes.bfloat16}[mmdt]
    x = np.asarray(x, dtype=np.float32).astype(dt)
    w_qkv = np.asarray(w_qkv, dtype=np.float32).astype(dt)
    w_out = np.asarray(w_out, dtype=np.float32).astype(wo_np)
    in_maps = []
    for c in range(N_CORES):
        b, hh = divmod(c, 2)
        hsl = slice(hh * HD, (hh + 1) * HD)
        in_maps.append({
            "xT": np.ascontiguousarray(x[b].T),
            "wqT": np.ascontiguousarray(w_qkv[0 * E:1 * E][hsl].T),
            "wkT": np.ascontiguousarray(w_qkv[1 * E:2 * E][hsl].T),
            "wvT": np.ascontiguousarray(w_qkv[2 * E:3 * E][hsl].T),
            "woT": np.ascontiguousarray(w_out[:, hsl].T),
        })
    return in_maps


def combine_outputs(results):
    y = np.empty((B, NT, E), dtype=np.float32)
    for b in range(B):
        y[b] = results[2 * b]["y"] + results[2 * b + 1]["y"]
    return y


def kernel(x, w_qkv, w_out):
    from concourse.bass_utils import run_bass_kernel_spmd
    nc = _get_nc()
    in_maps = make_in_maps(x, w_qkv, w_out)
    res = run_bass_kernel_spmd(nc, in_maps, core_ids=list(range(N_CORES)))
    return combine_outputs(res.results)



# revision 38
# speedup vs baseline: 1.5588x; 1.2720x over previous
"""Multi-head attention (dense transformer block) for 8 Trainium2 NeuronCores.

Problem: x [4, 2048, 1024] f32, w_qkv [3072, 1024], w_out [1024, 1024]
  qkv = x @ w_qkv.T ; split q,k,v ; 16 heads x 64 dims
  out = softmax(q k^T / 8) v ; y = out @ w_out.T

Sharding: 8 shards = (batch b in 0..3) x (head-half hh in 0..1).
Each core handles one batch and 8 heads end-to-end: QKV projection
column-split, attention for its 8 heads, out-projection row-split ->
partial y. Host sums the two partial y's per batch. No collectives.

Kernel structure (engines run their instruction streams in order, so the
phases are emitted as a software pipeline over head pairs):

    qk(0) | v | B(0) qk(1) C(0) | B(1) qk(2) C(1) | ... | B(3) C(3)

  - qk(p): q^T,k^T [128, tok] for pair p (fp32r matmuls, rotating bufs)
  - v: value projection -> vaug bf16 [ktok, head, 65] with a ones column
  - B(p): attention. Scores computed transposed per head S^T[ktok, qtok]
    with the two heads PAIRED via PE row-tiling (K=64 at partitions
    0/64) into adjacent PSUM banks; one ScalarE exp ACTIVATE [128, 1024]
    per k-tile covers both heads with the 1/8 scale folded in (softmax
    max-subtraction skipped; scores are O(+-6)). AV matmuls in bf16 with
    M=65: the ones column makes PSUM row 64 the softmax denominators.
    Normalization: DVE reciprocal -> GpSimd partition-broadcast -> DVE
    multiply (PE stays out of the softmax epilogue).
  - C(p): per-pair out-projection (K=128), accumulated into y in DRAM
    (first pair writes, later pairs DMA-accumulate).
"""

import numpy as np

B = 4
NT = 2048          # tokens per batch
E = 1024           # embed dim
H = 16             # heads
DH = 64            # head dim
HD = 512           # head dims per core (8 heads)
N_CORES = 8
SCALE = DH ** -0.5
P = 128

# DVE Schraudolph fast-exp: bf16 bits of exp(SCALE*s) ~= int16(s*FE_S + FE_B)
# (bf16 = 8-bit exponent + 7-bit mantissa; linear-mantissa approx, +-3% rel
# err, bias cancels in softmax). Lets the DVE take half the softmax exps.
import math
FE_S = SCALE * 128.0 / math.log(2.0)
FE_B = 127.0 * 128.0 - 5.5

_cache = {}


def _build(rep=1, ablate=(), mmdt="f32r", loop=False):
    import concourse.mybir as mybir
    import concourse.tile as tile
    from concourse import bacc
    from contextlib import ExitStack

    # dtype scheme: f32r/bf16/fp16 uniform; "mix" = fp16 q/k path + bf16 soft path
    f32 = mybir.dt.float32
    _qk = {"f32r": mybir.dt.float32r, "bf16": mybir.dt.bfloat16,
           "fp16": mybir.dt.float16, "mix": mybir.dt.float16}
    _soft = {"f32r": mybir.dt.bfloat16, "bf16": mybir.dt.bfloat16,
             "fp16": mybir.dt.float16, "mix": mybir.dt.bfloat16}
    f32r = _qk[mmdt]          # q/k-side matmul dtype (x, wq, wk, wv, qT, kT)
    bf16 = _soft[mmdt]        # softmax/out-side dtype (es, vaug, outT, woT)
    in_dt = {"f32r": f32, "bf16": mybir.dt.bfloat16,
             "fp16": mybir.dt.float16, "mix": mybir.dt.float16}[mmdt]
    wo_dt = {"f32r": f32, "bf16": mybir.dt.bfloat16,
             "fp16": mybir.dt.float16, "mix": mybir.dt.bfloat16}[mmdt]
    Exp = mybir.ActivationFunctionType.Exp
    Add = mybir.AluOpType.add

    nc = bacc.Bacc("TRN2", target_bir_lowering=False, debug=False,
                   enable_asserts=False, num_devices=N_CORES)

    xT_ap = nc.dram_tensor("xT", [E, NT], in_dt, kind="ExternalInput").ap()
    wqT_ap = nc.dram_tensor("wqT", [E, HD], in_dt, kind="ExternalInput").ap()
    wkT_ap = nc.dram_tensor("wkT", [E, HD], in_dt, kind="ExternalInput").ap()
    wvT_ap = nc.dram_tensor("wvT", [E, HD], in_dt, kind="ExternalInput").ap()
    woT_ap = nc.dram_tensor("woT", [HD, E], wo_dt, kind="ExternalInput").ap()
    y_ap = nc.dram_tensor("y", [NT, E], f32, kind="ExternalOutput").ap()

    KE = E // P        # 8 contraction tiles over embed
    MQ = HD // P       # 4 partition tiles over head dims = head pairs
    TQ = NT // 512     # 4 query chunks of 512
    TT = NT // P       # 16 token tiles of 128

    from concourse.tile_rust import add_dep_helper

    with tile.TileContext(nc) as tc, ExitStack() as ctx:
        per = ctx.enter_context(tc.tile_pool(name="per", bufs=1))
        qk_pool = ctx.enter_context(tc.tile_pool(name="qk", bufs=2))
        outT_pool = ctx.enter_context(tc.tile_pool(name="ot", bufs=5))
        es_pool = ctx.enter_context(tc.tile_pool(name="es", bufs=4))
        y_pool = ctx.enter_context(tc.tile_pool(name="ysb", bufs=2))
        nrm_pool = ctx.enter_context(tc.tile_pool(name="nrm", bufs=2))
        bcs_pool = ctx.enter_context(tc.tile_pool(name="bcs", bufs=2))
        xT_pool = ctx.enter_context(tc.tile_pool(name="xTp", bufs=2))
        psS = ctx.enter_context(tc.tile_pool(name="psS", bufs=2, space="PSUM"))
        psAV = ctx.enter_context(tc.tile_pool(name="psAV", bufs=2, space="PSUM"))
        psM = ctx.enter_context(tc.tile_pool(name="psM", bufs=2, space="PSUM"))

        # rep-invariant weights (wv first: the value projection runs first)
        wv = per.tile([P, KE, HD], f32r, tag="wv")
        nc.scalar.dma_start(wv[:], wvT_ap.rearrange("(o p) m -> p o m", p=P).bitcast(f32r))
        wq = per.tile([P, KE, HD], f32r, tag="wq")
        nc.scalar.dma_start(wq[:], wqT_ap.rearrange("(o p) m -> p o m", p=P).bitcast(f32r))
        wk = per.tile([P, KE, HD], f32r, tag="wk")
        nc.scalar.dma_start(wk[:], wkT_ap.rearrange("(o p) m -> p o m", p=P).bitcast(f32r))
        woT = per.tile([P, MQ, E], bf16, tag="woT")
        nc.scalar.dma_start(woT[:], woT_ap.rearrange("(o p) e -> p o e", p=P).bitcast(bf16))
        # double-buffered value tiles: rep r uses parity r % 2 so the next
        # rep's value projection can run as filler inside this rep.
        # Each head's slice is read by the AV matmul through a 128-column
        # window (65 real columns + overrun into the next head; 63 pad
        # columns after the last head) so every LDWEIGHTS is a full
        # 128-column load — that enables FWL + background-buffer overlap,
        # hiding the weight load under the previous AV matmul.
        VW = 8 * (DH + 1)          # 520 real columns per token row
        vaug_sets = []
        for par in range(2):
            vg = [per.tile([P, 4, VW + 63], bf16, tag=f"vaug{par}_{g}",
                           name=f"vaug{par}_{g}") for g in range(TT // 4)]
            for g in range(TT // 4):
                hv = vg[g][:, :, 0:VW].rearrange("p a (h c) -> p a h c", h=8)
                nc.vector.memset(hv[:, :, :, DH:DH + 1], 1.0)
            vaug_sets.append([vg[t // 4][:, t % 4] for t in range(TT)])

        # Tile does not order DMAs by DRAM range: chain each y region's
        # write DMAs explicitly across reps.
        y_prev_dma = {}
        # deferred work queue: (pe_weight, closure) for out-projection
        # chunks and next-rep value-projection groups. Items drain as
        # kt-loop filler across pair and rep boundaries so this work rides
        # in the exp-gated PE bubbles instead of forming serial phases.
        work_queue = []

        def pop_filler(budget):
            items = []
            while work_queue and budget > 0:
                w, c = work_queue[0]
                if w > budget and items:
                    break
                work_queue.pop(0)
                items.append(c)
                budget -= w
            return items

        xT_src = xT_ap.rearrange("(o p) t -> p o t", p=P).bitcast(f32r)

        def emit_xT_dmas(gen):
            # on the GpSimd DMA queue: the sync queue carries the y writes
            # and an xT load would head-block them for ~3us each.
            xTs = []
            for ke in range(KE):
                xk = xT_pool.tile([P, NT], f32r, tag=f"xT{ke}",
                                  name=f"xT{ke}g{gen}")
                nc.gpsimd.dma_start(xk[:], xT_src[:, ke, :])
                xTs.append(xk)
            return xTs

        def emit_qk_group(xTs, mq, dst, w, tq, rot=0):
            ps = psM.tile([P, 512], f32, tag="m")
            for i in range(KE):
                ke = (i + rot) % KE
                nc.tensor.matmul(ps[:], w[:, ke, mq * P:(mq + 1) * P],
                                 xTs[ke][:, tq * 512:(tq + 1) * 512],
                                 start=(i == 0), stop=(i == KE - 1))
            nc.vector.tensor_copy(dst[:, tq * 512:(tq + 1) * 512], ps[:])

        def alloc_qk(mq):
            qT = qk_pool.tile([P, NT], f32r, tag="qTp", name=f"qT{mq}")
            kT = qk_pool.tile([P, NT], f32r, tag="kTp", name=f"kT{mq}")
            return qT, kT

        def qk_groups(mq, qT, kT):
            for dst, w in ((kT, wk), (qT, wq)):
                for tq in range(TQ):
                    yield (mq, dst, w, tq)

        def emit_v_group(xTs, vaugs, tt):
            ps = psM.tile([P, HD], f32, tag="m")
            for i in range(KE):
                ke = (i + tt) % KE
                nc.tensor.matmul(ps[:], xTs[ke][:, tt * P:(tt + 1) * P],
                                 wv[:, ke, :], start=(i == 0), stop=(i == KE - 1))
            dst = vaugs[tt][:, 0:VW].rearrange("p (h c) -> p h c", h=8)
            nc.vector.tensor_copy(dst[:, :, 0:DH],
                                  ps[:].rearrange("p (h d) -> p h d", h=8))

        def emit_attn_tq(vaugs, pair, qT, kT, outT, tq, filler=()):
            filler = list(filler)
            qsl = slice(tq * 512, (tq + 1) * 512)
            av0 = psAV.tile([DH + 1, 512], f32, tag="av")
            av1 = psAV.tile([DH + 1, 512], f32, tag="av")

            def emit_av(kt, es):
                h0, h1 = 2 * pair, 2 * pair + 1
                nc.tensor.matmul(
                    av0[:], vaugs[kt][:, h0 * (DH + 1):(h0 + 1) * (DH + 1)],
                    es[:, 0, :], start=(kt == 0), stop=(kt == TT - 1))
                nc.tensor.matmul(
                    av1[:], vaugs[kt][:, h1 * (DH + 1):(h1 + 1) * (DH + 1)],
                    es[:, 1, :], start=(kt == 0), stop=(kt == TT - 1))

            # AV lags scores/exp by two k-tiles so the PE never sits in
            # the scores -> exp -> AV semaphore chain: while ScalarE
            # exps tile kt, the PE already runs scores of kt+1/kt+2, and
            # a late exp (e.g. behind a filler lump) never stalls the PE.
            pending = []
            for kt in range(TT):
                # drain filler work spread across the kt loop: the
                # exp-gated PE bubbles absorb the matmuls and the DVE
                # copies interleave with the loop instead of ganging up
                # at tq boundaries.
                if filler:
                    for _ in range(-(-len(filler) // (TT - kt))):
                        filler.pop(0)()
                ksl = slice(kt * P, (kt + 1) * P)
                sps = psS.tile([P, 2, 512], f32, tag="s")
                nc.tensor.matmul(sps[:, 0, :], kT[0:DH, ksl],
                                 qT[0:DH, qsl], start=True, stop=True)
                nc.tensor.matmul(sps[:, 1, :], kT[DH:P, ksl],
                                 qT[DH:P, qsl], start=True, stop=True)
                if "exp" in ablate:
                    continue
                es = es_pool.tile([P, 2, 512], bf16, tag="es")
                nc.scalar.activation(es[:], sps[:], Exp, scale=SCALE)
                if "av" in ablate:
                    continue
                pending.append((kt, es))
                if len(pending) > 2:
                    emit_av(*pending.pop(0))
            for f in filler:
                f()
            if "av" in ablate or "exp" in ablate:
                return
            for p in pending:
                emit_av(*p)
            for j, av in ((0, av0), (1, av1)):
                # custom-DVE recip requires matching in/out base
                # partitions; den sits at PSUM partition 64, so hop it
                # to partition 0 first (stock copy handles the shift).
                den = nrm_pool.tile([1, 512], f32, tag="den")
                nc.vector.tensor_copy(den[:], av[DH:DH + 1, :])
                recip = nrm_pool.tile([1, 512], f32, tag="recip")
                nc.vector.reciprocal_approx_fast(recip[:], den[:])
                bcs = bcs_pool.tile([DH, 512], f32, tag="bcs")
                nc.gpsimd.partition_broadcast(bcs[:], recip[:])
                nc.vector.tensor_tensor(
                    outT[j * DH:(j + 1) * DH, qsl],
                    av[0:DH, :], bcs[:], mybir.AluOpType.mult)

        def emit_outproj_chunk(outTs, tt, ec):
            # all four pairs' contributions accumulated in one PSUM
            # group, then a single copy + write DMA per y region.
            # ps lives in psM (not psAV) so outproj matmuls never wait
            # on the softmax epilogue's reads of the av tiles.
            esl = slice(ec * 512, (ec + 1) * 512)
            ps = psM.tile([P, 512], f32, tag="m")
            for pr in range(MQ):
                nc.tensor.matmul(ps[:], outTs[pr][:, tt * P:(tt + 1) * P],
                                 woT[:, pr, esl],
                                 start=(pr == 0), stop=(pr == MQ - 1))
            ysb = y_pool.tile([P, 512], f32, tag="ysb")
            nc.vector.tensor_copy(ysb[:], ps[:])
            dma = nc.sync.dma_start(y_ap[tt * P:(tt + 1) * P, esl], ysb[:])
            if (tt, ec) in y_prev_dma:
                add_dep_helper(dma.ins, y_prev_dma[(tt, ec)].ins,
                               reason="y write order across reps")
            y_prev_dma[(tt, ec)] = dma

        def emit_body(pre, nxt_gen):
            """One rep: attention pairs 0-3 using tiles prepared by the
            previous rep's pipeline, while preparing the next rep's
            inputs (xT DMAs at pair 0, value groups queued at pair 2,
            qk(0) as pair 3's direct filler)."""
            xTs, vaugs, qT, kT = pre
            nxt_xTs = nxt_vaugs = nxt_qT = nxt_kT = None
            outTs = []
            for pair in range(MQ):
                outT = outT_pool.tile([P, NT], bf16, tag="outT", name=f"outT{pair}")
                outTs.append(outT)
                if pair == 0 and nxt_gen is not None:
                    nxt_xTs = emit_xT_dmas(nxt_gen)
                    nxt_vaugs = vaug_sets[nxt_gen % 2]
                if pair == 2 and nxt_gen is not None:
                    for tt in range(TT):
                        work_queue.append(
                            (2, lambda x=nxt_xTs, v=nxt_vaugs, tt=tt:
                             emit_v_group(x, v, tt)))
                if pair + 1 < MQ:
                    nqT, nkT = alloc_qk(pair + 1)
                    qk_iter = qk_groups(pair + 1, nqT, nkT)
                    qk_xTs = xTs
                elif nxt_gen is not None:
                    # pair 3's direct filler is the NEXT rep's qk(0)
                    nxt_qT, nxt_kT = alloc_qk(0)
                    qk_iter = qk_groups(0, nxt_qT, nxt_kT)
                    qk_xTs = nxt_xTs
                else:
                    nqT = nkT = None
                    qk_iter = iter(())
                    qk_xTs = xTs
                for tq in range(TQ):
                    filler = pop_filler(6)
                    for _ in range(2):
                        g = next(qk_iter, None)
                        if g is not None:
                            filler.append(
                                lambda g=g, x=qk_xTs: emit_qk_group(x, *g))
                    if "scores" not in ablate:
                        emit_attn_tq(vaugs, pair, qT, kT, outT, tq, filler)
                    else:
                        for f in filler:
                            f()
                    if "outproj" not in ablate and pair == MQ - 1:
                        # this tq's tokens are now complete across all four
                        # pairs: queue their out-projection chunks.
                        for tt in range(tq * 4, tq * 4 + 4):
                            for ec in range(E // 512):
                                work_queue.append(
                                    (1, lambda o=list(outTs), tt=tt, ec=ec:
                                     emit_outproj_chunk(o, tt, ec)))
                if pair + 1 < MQ:
                    qT, kT = nqT, nkT
            return (nxt_xTs, nxt_vaugs, nxt_qT, nxt_kT)

        def emit_prologue(gen):
            # unpipelined lead-in for the first rep (and the loop path)
            xTs = emit_xT_dmas(gen)
            vaugs = vaug_sets[gen % 2]
            for tt in range(TT):
                emit_v_group(xTs, vaugs, tt)
            qT, kT = alloc_qk(0)
            for gi, g in enumerate(qk_groups(0, qT, kT)):
                emit_qk_group(xTs, *g, rot=gi)
            return (xTs, vaugs, qT, kT)

        def drain_queue():
            for w, f in work_queue:
                f()
            work_queue.clear()

        if loop:
            with tc.For_i(0, rep, 1):
                pre = emit_prologue(0)
                emit_body(pre, None)
                drain_queue()
        else:
            pre = emit_prologue(0)
            for r in range(rep):
                pre = emit_body(pre, r + 1 if r + 1 < rep else None)
            drain_queue()

    nc.compile()
    return nc


MMDT = "bf16"


def _get_nc(rep=1, ablate=(), mmdt=None):
    mmdt = mmdt or MMDT
    key = ("nc", rep, tuple(sorted(ablate)), mmdt)
    if key not in _cache:
        _cache[key] = _build(rep, ablate, mmdt)
    return _cache[key]


def make_in_maps(x, w_qkv, w_out, mmdt=None):
    import ml_dtypes
    mmdt = mmdt or MMDT
    dt = {"f32r": np.float32, "bf16": ml_dtypes.bfloat16,
          "fp16": np.float16, "mix": np.float16}[mmdt]
    wo_np = {"f32r": np.float32, "bf16": ml_dtypes.bfloat16,
             "fp16": np.float16, "mix": ml_dtypes.bfloat16}[mmdt]
    x = np.asarray(x, dtype=np.float32).astype(dt)
    w_qkv = np.asarray(w_qkv, dtype=np.float32).astype(dt)
    w_out = np.asarray(w_out, dtype=np.float32).astype(wo_np)
    in_maps = []
    for c in range(N_CORES):
        b, hh = divmod(c, 2)
        hsl = slice(hh * HD, (hh + 1) * HD)
        in_maps.append({
            "xT": np.ascontiguousarray(x[b].T),
            "wqT": np.ascontiguousarray(w_qkv[0 * E:1 * E][hsl].T),
            "wkT": np.ascontiguousarray(w_qkv[1 * E:2 * E][hsl].T),
            "wvT": np.ascontiguousarray(w_qkv[2 * E:3 * E][hsl].T),
            "woT": np.ascontiguousarray(w_out[:, hsl].T),
        })
    return in_maps


def combine_outputs(results):
    y = np.empty((B, NT, E), dtype=np.float32)
    for b in range(B):
        y[b] = results[2 * b]["y"] + results[2 * b + 1]["y"]
    return y


def kernel(x, w_qkv, w_out):
    from concourse.bass_utils import run_bass_kernel_spmd
    nc = _get_nc()
    in_maps = make_in_maps(x, w_qkv, w_out)
    res = run_bass_kernel_spmd(nc, in_maps, core_ids=list(range(N_CORES)))
    return combine_outputs(res.results)

